# revision 1
# baseline (speedup 1.0000x reference)
"""Trainium2 Bass kernel for a dense transformer block (B=2, T=2048, C=1024, nh=16, H=4096).

Strategy (8 NeuronCores, no device collectives -- they measure ~300us for an 8MB
quad AllReduce here, far more than the whole compute budget):

  Launch 1 (head-parallel): cores 0-3 <- batch 0, cores 4-7 <- batch 1; each core
    handles 4 attention heads over the full sequence. x arrives pre-transposed
    (feature-major); LN1 statistics are computed with ones-vector matmuls on the
    TensorE and applied with broadcast vector ops, so no on-device transposes are
    needed. QKV run as fp32r matmuls (feature-major Q/K, token-major V with a
    ones column appended so the softmax denominator falls out of the AV matmul).
    Causal attention uses 512-token query chunks with the key-tile loop outer;
    exp on ScalarE, diagonal-block multiplicative masks, denominator divide on
    VectorE after a GpSimd partition broadcast. Output: row-parallel c_proj
    partial [2048, 1024].

  Host: pure re-slicing of the partials (no arithmetic).

  Launch 2 (token-parallel): each core takes a 512-token slice: sums the 4 proj
    partials on-device, + residual + proj_b -> LN2 -> c_fc (feature-major hidden)
    -> gaussian activation (2 ScalarE passes; mu/sigma/fc_b folded into the
    activation bias/scale, gamma/beta folded into fc2 weights/bias on host)
    -> c_fc2 -> + residual -> final output slice. MLP weights stream from HBM in
    contiguous per-chunk layouts to keep the DMA engines at line rate.

Both launches are uniform SPMD programs (same instruction stream on all 8 cores,
different data), run via run_bass_kernel_spmd.
"""

import hashlib
import os
import shutil
from contextlib import ExitStack

import numpy as np

import concourse.bass as bass
import concourse.tile as tile
from concourse import bacc, mybir
from concourse.bass_utils import run_bass_kernel_spmd

F32 = mybir.dt.float32
F32R = mybir.dt.float32r
AF = mybir.ActivationFunctionType
ALU = mybir.AluOpType

N_CORES = 8
T = 2048          # tokens per batch
C = 1024          # model dim
NH_LOC = 4        # heads per core (launch 1)
HS = 64           # head size
HID = 4096        # mlp hidden
TS = 512          # tokens per core (launch 2)

LAST_EXEC_NS = {}  # launch name -> exec_time_ns (filled when tracing enabled)

_CACHE_DIR = "/tmp/neff_cache"


def _install_compile_cache():
    import concourse.bass2jax as b2j

    if getattr(b2j, "_neff_cache_installed", False):
        return
    real = b2j.compile_bir_kernel

    def cached(bir_json, tmpdir, neff_name="file.neff"):
        os.makedirs(_CACHE_DIR, exist_ok=True)
        h = hashlib.sha256(bir_json).hexdigest()
        cpath = os.path.join(_CACHE_DIR, h + ".neff")
        out = os.path.join(tmpdir, neff_name)
        if os.path.exists(cpath):
            shutil.copyfile(cpath, out)
            return out
        res = real(bir_json, tmpdir, neff_name)
        shutil.copyfile(res, cpath)
        return res

    b2j.compile_bir_kernel = cached
    b2j._neff_cache_installed = True


# --------------------------------------------------------------------------
# Launch 1: LN1 + QKV + causal attention (4 heads) + c_proj partial
# --------------------------------------------------------------------------
def build_l1():
    nc = bacc.Bacc("TRN2", target_bir_lowering=False, debug=False,
                   num_devices=N_CORES)
    xT_d = nc.dram_tensor("xT", [4, 128, 8, 512], F32R, kind="ExternalInput")
    wqk_d = nc.dram_tensor("wqkT", [128, 8, 512], F32R, kind="ExternalInput")
    wv_d = nc.dram_tensor("wvT", [128, 8, 256], F32R, kind="ExternalInput")
    bqk_d = nc.dram_tensor("bqk", [128, 4], F32, kind="ExternalInput")
    bv_d = nc.dram_tensor("bv", [1, 256], F32, kind="ExternalInput")
    wsqk_d = nc.dram_tensor("wsqk", [128, 4], F32, kind="ExternalInput")
    nwsv_d = nc.dram_tensor("nwsv", [1, 256], F32, kind="ExternalInput")
    pw_d = nc.dram_tensor("projwT", [128, 2, 1024], F32R, kind="ExternalInput")
    mask_d = nc.dram_tensor("masks", [128, 4, 512], F32R, kind="ExternalInput")
    vones_d = nc.dram_tensor("vones", [128, 64], F32R, kind="ExternalInput")
    id_d = nc.dram_tensor("ident", [128, 128], F32, kind="ExternalInput")
    yp_d = nc.dram_tensor("yp", [T, C], F32, kind="ExternalOutput")

    n_ttiles = T // 128          # 16
    n_tc = T // 512              # 4 attention query chunks

    with tile.TileContext(nc) as tc, ExitStack() as ctx:
        consts = ctx.enter_context(tc.tile_pool(name="consts", bufs=1))
        vones_sb = consts.tile([128, 64], F32R)
        nc.sync.dma_start(out=vones_sb[:], in_=vones_d[:])
        eps_sb = consts.tile([128, 1], F32)
        nc.vector.memset(eps_sb[:], 1e-5)
        bqk_sb = consts.tile([128, 4], F32)
        nc.sync.dma_start(out=bqk_sb[:], in_=bqk_d[:])
        wsqk_sb = consts.tile([128, 4], F32)
        nc.sync.dma_start(out=wsqk_sb[:], in_=wsqk_d[:])
        ident = consts.tile([128, 128], F32)
        nc.sync.dma_start(out=ident[:], in_=id_d[:])
        bv_row = consts.tile([1, 256], F32)
        nc.sync.dma_start(out=bv_row[:], in_=bv_d[:])
        bv_b = consts.tile([128, 256], F32)
        nc.gpsimd.partition_broadcast(bv_b[:], bv_row[:])
        nwsv_row = consts.tile([1, 256], F32)
        nc.sync.dma_start(out=nwsv_row[:], in_=nwsv_d[:])
        nwsv_b = consts.tile([128, 256], F32)
        nc.gpsimd.partition_broadcast(nwsv_b[:], nwsv_row[:])

        big2 = ctx.enter_context(tc.tile_pool(name="big2", bufs=1))
        qkT = big2.tile([128, 4, T], F32R)     # Q feats (tiles 0,1), K feats (2,3)
        v_sb = big2.tile([128, n_ttiles, NH_LOC, 65], F32R)  # col 64 = ones
        yT = big2.tile([128, 2, T], F32R)

        wpool = ctx.enter_context(tc.tile_pool(name="wpool", bufs=1))
        wqk_sb = wpool.tile([128, 8, 512], F32R)
        nc.sync.dma_start(out=wqk_sb[:], in_=wqk_d[:])
        wv_sb = wpool.tile([128, 8, 256], F32R)
        nc.sync.dma_start(out=wv_sb[:], in_=wv_d[:])

        # ---- P1+P2 fused per 512-token chunk: raw QKV matmuls on un-normalized
        # xT; the LayerNorm is applied inside the qkv epilogue:
        #   qkv[f,t] = r[t]*raw[f,t] - r[t]*mu[t]*rowsum(W)[f] + b[f]
        # so the TensorE never waits on the LN chain.
        with tc.tile_pool(name="p1", bufs=3) as p1, \
             tc.tile_pool(name="p1sl", bufs=2) as p1sl, \
             tc.tile_pool(name="p1b", bufs=2) as p1b, \
             tc.tile_pool(name="p1c", bufs=2) as p1c, \
             tc.tile_pool(name="p1r", bufs=8) as p1r, \
             tc.tile_pool(name="p1psum", bufs=1, space="PSUM") as p1p, \
             tc.tile_pool(name="ptp", bufs=2, space="PSUM") as ptp, \
             tc.tile_pool(name="p2psum", bufs=2, space="PSUM") as p2p:
            for tch in range(n_tc):
                sl = slice(tch * 512, (tch + 1) * 512)
                slab = p1sl.tile([128, 8, 512], F32R, tag="slab")
                nc.sync.dma_start(out=slab[:], in_=xT_d[tch])
                ps_sum = p1p.tile([1, 512], F32, tag="s")
                ps_sq = p1p.tile([1, 512], F32, tag="q")
                for c in range(8):
                    sq = p1.tile([128, 512], F32R, tag="sq")
                    nc.scalar.activation(out=sq[:], in_=slab[:, c, :],
                                         func=AF.Square)
                    nc.tensor.matmul(ps_sum[:], vones_sb[:, 0:1], slab[:, c, :],
                                     start=(c == 0), stop=(c == 7),
                                     skip_group_check=True)
                    nc.tensor.matmul(ps_sq[:], vones_sb[:, 0:1], sq[:],
                                     start=(c == 0), stop=(c == 7),
                                     skip_group_check=True)
                # row math: mu, rstd, r*mu
                srow = p1r.tile([1, 512], F32, tag="row")
                nc.vector.tensor_copy(srow[:], ps_sum[:])
                qrow = p1r.tile([1, 512], F32, tag="row")
                nc.vector.tensor_copy(qrow[:], ps_sq[:])
                mrow = p1r.tile([1, 512], F32, tag="row")
                nc.vector.tensor_scalar(out=mrow[:], in0=srow[:], scalar1=1.0 / C,
                                        scalar2=None, op0=ALU.mult)
                msq = p1r.tile([1, 512], F32, tag="row")
                nc.vector.tensor_mul(msq[:], mrow[:], mrow[:])
                vrow = p1r.tile([1, 512], F32, tag="row")
                nc.vector.scalar_tensor_tensor(out=vrow[:], in0=qrow[:],
                                               scalar=1.0 / C, in1=msq[:],
                                               op0=ALU.mult, op1=ALU.subtract)
                sdr = p1r.tile([1, 512], F32, tag="row")
                nc.scalar.activation(out=sdr[:], in_=vrow[:], func=AF.Sqrt,
                                     bias=eps_sb[0:1], scale=1.0)
                rrow = p1r.tile([1, 512], F32, tag="row")
                nc.vector.reciprocal(rrow[:], sdr[:])
                rmurow = p1r.tile([1, 512], F32, tag="row")
                nc.vector.tensor_mul(rmurow[:], rrow[:], mrow[:])
                rb = p1b.tile([128, 512], F32, tag="rb")
                nc.gpsimd.partition_broadcast(rb[:], rrow[:])
                rmu_b = p1b.tile([128, 512], F32, tag="rmu")
                nc.gpsimd.partition_broadcast(rmu_b[:], rmurow[:])
                # token-major per-partition columns of r and r*mu (for V)
                rcol = p1c.tile([128, 4], F32, tag="rcol")
                rmucol = p1c.tile([128, 4], F32, tag="rmucol")
                for j in range(4):
                    tp1 = ptp.tile([128, 1], F32, tag="tp")
                    nc.tensor.transpose(tp1[:], rrow[0:1, j * 128:(j + 1) * 128],
                                        ident[0:1, 0:1])
                    nc.vector.tensor_copy(rcol[:, j:j + 1], tp1[:])
                    tp2 = ptp.tile([128, 1], F32, tag="tp")
                    nc.tensor.transpose(tp2[:], rmurow[0:1, j * 128:(j + 1) * 128],
                                        ident[0:1, 0:1])
                    nc.vector.tensor_copy(rmucol[:, j:j + 1], tp2[:])
                # raw Q/K projections + LN epilogue
                for f in range(4):
                    ps = p2p.tile([128, 512], F32, tag="qk")
                    for c in range(8):
                        nc.tensor.matmul(
                            ps[:], wqk_sb[:, c, f * 128:(f + 1) * 128],
                            slab[:, c, :], start=(c == 0), stop=(c == 7))
                    t1 = p1.tile([128, 512], F32, tag="t1")
                    nc.vector.tensor_mul(t1[:], ps[:], rb[:])
                    t2 = p1.tile([128, 512], F32, tag="t2")
                    nc.vector.tensor_scalar(out=t2[:], in0=rmu_b[:],
                                            scalar1=wsqk_sb[:, f:f + 1],
                                            scalar2=bqk_sb[:, f:f + 1],
                                            op0=ALU.mult, op1=ALU.subtract)
                    nc.vector.tensor_sub(qkT[:, f, sl], t1[:], t2[:])
                # raw V + LN epilogue
                for tt4 in range(4):
                    tt = tch * 4 + tt4
                    ps = p2p.tile([128, 256], F32, tag="v")
                    for c in range(8):
                        nc.tensor.matmul(
                            ps[:], slab[:, c, tt4 * 128:(tt4 + 1) * 128],
                            wv_sb[:, c, :], start=(c == 0), stop=(c == 7))
                    t1v = p1.tile([128, 256], F32, tag="t1v")
                    nc.vector.tensor_scalar(out=t1v[:], in0=ps[:],
                                            scalar1=rcol[:, tt4:tt4 + 1],
                                            scalar2=None, op0=ALU.mult)
                    t1b = p1.tile([128, 256], F32, tag="t1b")
                    nc.gpsimd.tensor_add(t1b[:], t1v[:], bv_b[:])
                    nc.vector.scalar_tensor_tensor(
                        out=v_sb[:, tt, :, 0:64],
                        in0=nwsv_b[:].rearrange("p (h d) -> p h d", h=NH_LOC),
                        scalar=rmucol[:, tt4:tt4 + 1],
                        in1=t1b[:].rearrange("p (h d) -> p h d", h=NH_LOC),
                        op0=ALU.mult, op1=ALU.add)
            nc.sync.dma_start(out=v_sb[:, :, :, 64:65], in_=vones_d[:])

        late = ctx.enter_context(tc.tile_pool(name="late", bufs=1))
        pw_sb = late.tile([128, 2, 1024], F32R)
        nc.sync.dma_start(out=pw_sb[:], in_=pw_d[:])

        # ---- P3: attention, software-pipelined (QK for s+1 before AV of s) ----
        with tc.tile_pool(name="p3consts", bufs=1) as p3c, \
             tc.tile_pool(name="p3a", bufs=6) as p3a, \
             tc.tile_pool(name="p3s", bufs=3) as p3s, \
             tc.tile_pool(name="p3ps2", bufs=2, space="PSUM") as p3ps2, \
             tc.tile_pool(name="p3ps1", bufs=2, space="PSUM") as p3ps1, \
             tc.tile_pool(name="p3py", bufs=1, space="PSUM") as p3py:
            mask_sb = p3c.tile([128, 4, 512], F32R)
            nc.sync.dma_start(out=mask_sb[:], in_=mask_d[:])
            yc_all = p3c.tile([65, 16, 512], F32)
            dnpack = p3c.tile([16, 512], F32)
            rpack = p3c.tile([16, 512], F32)
            for h in range(NH_LOC):
                po = (h % 2) * 64
                qf = h // 2
                kf = 2 + h // 2
                for pair in ((0, 1), (2, 3)):
                    smax = 4 * pair[1] + 4
                    pys = {}
                    for tcx in pair:
                        py_t = p3py.tile([65, 512], F32, tag=f"py{tcx % 2}",
                                         name=f"py{h}_{tcx}")
                        pys[tcx] = py_t

                    def emit_qk(s):
                        tiles = []
                        tcs = [tcx for tcx in pair if s <= 4 * tcx + 3]
                        if not tcs:
                            return tiles
                        wide = len(tcs) == 2
                        pool = p3ps2 if wide else p3ps1
                        pscore = pool.tile([128, 1024 if wide else 512], F32,
                                           tag="sc2" if wide else "sc1",
                                           name=f"sc{h}_{s}_{tcs[0]}")
                        for i, tcx in enumerate(tcs):
                            qsl = slice(tcx * 512, (tcx + 1) * 512)
                            nc.tensor.matmul(
                                pscore[:, i * 512:(i + 1) * 512],
                                qkT[po:po + 64, kf, s * 128:(s + 1) * 128],
                                qkT[po:po + 64, qf, qsl],
                                start=True, stop=True, skip_group_check=True)
                        at = p3a.tile([128, 1024 if wide else 512], F32R,
                                      tag="at2" if wide else "at1",
                                      name=f"at{h}_{s}_{tcs[0]}")
                        nc.scalar.activation(out=at[:], in_=pscore[:],
                                             func=AF.Exp)
                        for i, tcx in enumerate(tcs):
                            if tcx == s // 4:
                                nc.vector.tensor_mul(
                                    at[:, i * 512:(i + 1) * 512],
                                    at[:, i * 512:(i + 1) * 512],
                                    mask_sb[:, s % 4, :])
                            tiles.append((tcx, at[:, i * 512:(i + 1) * 512]))
                        return tiles

                    cur = emit_qk(0)
                    for s in range(smax):
                        nxt = emit_qk(s + 1) if s + 1 < smax else []
                        for tcx, atv in cur:
                            nc.tensor.matmul(pys[tcx][:], v_sb[:, s, h, :], atv,
                                             start=(s == 0),
                                             stop=(s == 4 * tcx + 3),
                                             skip_group_check=True)
                            if s == 4 * tcx + 3:
                                idx = h * 4 + tcx
                                nc.vector.tensor_copy(yc_all[:, idx, :],
                                                      pys[tcx][:])
                                nc.sync.dma_start(
                                    out=dnpack[idx:idx + 1, :],
                                    in_=yc_all[64:65, idx, :])
                        cur = nxt
            # batched softmax denominators: one reciprocal for all 16 rows
            nc.vector.reciprocal(rpack[:], dnpack[:])
            for h in range(NH_LOC):
                po = (h % 2) * 64
                for tcx in range(n_tc):
                    idx = h * 4 + tcx
                    qsl = slice(tcx * 512, (tcx + 1) * 512)
                    rrow1 = p3s.tile([1, 512], F32, tag="rr1")
                    nc.sync.dma_start(out=rrow1[:], in_=rpack[idx:idx + 1, :])
                    db = p3s.tile([64, 512], F32, tag="db")
                    nc.gpsimd.partition_broadcast(db[:], rrow1[:])
                    nc.vector.tensor_mul(
                        yT[po:po + 64, h // 2, qsl],
                        yc_all[0:64, idx, :], db[:])

        # ---- P4: c_proj partial ----
        with tc.tile_pool(name="p4o", bufs=3) as p4o, \
             tc.tile_pool(name="p4ps", bufs=2, space="PSUM") as p4p:
            for tt in range(n_ttiles):
                for co in range(2):
                    pp = p4p.tile([128, 512], F32)
                    for cl in range(2):
                        nc.tensor.matmul(
                            pp[:], yT[:, cl, tt * 128:(tt + 1) * 128],
                            pw_sb[:, cl, co * 512:(co + 1) * 512],
                            start=(cl == 0), stop=(cl == 1))
                    ot = p4o.tile([128, 512], F32)
                    nc.vector.tensor_copy(ot[:], pp[:])
                    nc.sync.dma_start(
                        out=yp_d[tt * 128:(tt + 1) * 128,
                                 co * 512:(co + 1) * 512], in_=ot[:])
    nc.compile()
    return nc


# --------------------------------------------------------------------------
# Launch 2: reduce partials + residual + LN2 + MLP + residual
# --------------------------------------------------------------------------
def build_l2(s_act: float):
    nc = bacc.Bacc("TRN2", target_bir_lowering=False, debug=False,
                   num_devices=N_CORES)
    yp4_d = nc.dram_tensor("yp4", [16, 128, C], F32R, kind="ExternalInput")
    gones_d = nc.dram_tensor("gones", [128, 4, 128], F32R, kind="ExternalInput")
    xs_d = nc.dram_tensor("xs", [TS, C], F32, kind="ExternalInput")
    pb_d = nc.dram_tensor("pb", [1, C], F32, kind="ExternalInput")
    fb2_d = nc.dram_tensor("fb2", [1, C], F32, kind="ExternalInput")
    ab_d = nc.dram_tensor("abias", [128, 32], F32, kind="ExternalInput")
    fcw_d = nc.dram_tensor("fcwT", [8, 128, 8, 512], F32R, kind="ExternalInput")
    fc2w_d = nc.dram_tensor("fc2wT", [8, 128, 4, C], F32R, kind="ExternalInput")
    id_d = nc.dram_tensor("ident", [128, 128], F32, kind="ExternalInput")
    out_d = nc.dram_tensor("out", [TS, C], F32, kind="ExternalOutput")

    n_ttiles = TS // 128    # 4

    with tile.TileContext(nc) as tc, ExitStack() as ctx:
        consts = ctx.enter_context(tc.tile_pool(name="consts", bufs=1))
        ident = consts.tile([128, 128], F32)
        nc.sync.dma_start(out=ident[:], in_=id_d[:])
        eps_sb = consts.tile([128, 1], F32)
        nc.vector.memset(eps_sb[:], 1e-5)
        pb_row = consts.tile([1, C], F32)
        nc.sync.dma_start(out=pb_row[:], in_=pb_d[:])
        pb_b = consts.tile([128, C], F32)
        nc.gpsimd.partition_broadcast(pb_b[:], pb_row[:])
        fb2_row = consts.tile([1, C], F32)
        nc.sync.dma_start(out=fb2_row[:], in_=fb2_d[:])
        fb2_b = consts.tile([128, C], F32)
        nc.gpsimd.partition_broadcast(fb2_b[:], fb2_row[:])
        ab_sb = consts.tile([128, 32], F32)
        nc.sync.dma_start(out=ab_sb[:], in_=ab_d[:])
        gones_sb = consts.tile([128, 4, 128], F32R)
        nc.sync.dma_start(out=gones_sb[:], in_=gones_d[:])

        big = ctx.enter_context(tc.tile_pool(name="big", bufs=1))
        h2T = big.tile([128, 8, TS], F32R)         # 16KB/p
        x2pb = big.tile([128, n_ttiles, C], F32)   # x2 + fc2 bias, 16KB/p
        actT = big.tile([128, 32, TS], F32R)       # 64KB/p

        # ---- P1: reduce partials, LN2, transpose ----
        with tc.tile_pool(name="q1", bufs=3) as q1, \
             tc.tile_pool(name="q1s", bufs=4) as q1s, \
             tc.tile_pool(name="q1psum", bufs=2, space="PSUM") as q1p:
            for tt in range(n_ttiles):
                x2 = q1.tile([128, C], F32, tag="x2")
                px2 = [q1p.tile([128, 512], F32, tag=f"px{ch}", name=f"px{tt}_{ch}")
                       for ch in range(2)]
                for j in range(4):
                    gt = q1.tile([128, C], F32R, tag="g")
                    nc.sync.dma_start(out=gt[:], in_=yp4_d[tt * 4 + j])
                    for ch in range(2):
                        nc.tensor.matmul(
                            px2[ch][:], gones_sb[:, j, :],
                            gt[:, ch * 512:(ch + 1) * 512],
                            start=(j == 0), stop=(j == 3),
                            skip_group_check=True)
                xst = q1.tile([128, C], F32, tag="xs")
                nc.sync.dma_start(out=xst[:], in_=xs_d[tt * 128:(tt + 1) * 128, :])
                xpb = q1.tile([128, C], F32, tag="xpb")
                nc.gpsimd.tensor_add(xpb[:], xst[:], pb_b[:])
                for ch in range(2):
                    csl = slice(ch * 512, (ch + 1) * 512)
                    nc.vector.tensor_add(x2[:, csl], px2[ch][:], xpb[:, csl])
                nc.vector.tensor_add(x2pb[:, tt, :], x2[:], fb2_b[:])
                stats = q1s.tile([128, 2, 6], F32)
                x2g = x2[:].rearrange("p (g d) -> p g d", g=2)
                nc.vector.bn_stats(out=stats[:, 0, :], in_=x2g[:, 0, :])
                nc.vector.bn_stats(out=stats[:, 1, :], in_=x2g[:, 1, :])
                mv = q1s.tile([128, 2], F32)
                nc.vector.bn_aggr(out=mv[:], in_=stats[:])
                sd = q1s.tile([128, 1], F32, tag="sd")
                nc.scalar.activation(out=sd[:], in_=mv[:, 1:2], func=AF.Sqrt,
                                     bias=eps_sb[:], scale=1.0)
                rstd = q1s.tile([128, 1], F32)
                nc.vector.reciprocal(rstd[:], sd[:])
                h2 = q1.tile([128, C], F32, tag="h2")
                nc.vector.tensor_scalar(out=h2[:], in0=x2[:],
                                        scalar1=mv[:, 0:1], scalar2=rstd[:],
                                        op0=ALU.subtract, op1=ALU.mult)
                for c in range(8):
                    pt = q1p.tile([128, 128], F32)
                    nc.tensor.transpose(pt[:], h2[:, c * 128:(c + 1) * 128], ident[:])
                    nc.vector.tensor_copy(h2T[:, c, tt * 128:(tt + 1) * 128], pt[:])

        # ---- P2: c_fc + gaussian activation (feature-major) ----
        with tc.tile_pool(name="q2w", bufs=2) as q2w, \
             tc.tile_pool(name="q2t", bufs=3) as q2t, \
             tc.tile_pool(name="q2psum", bufs=3, space="PSUM") as q2p:
            for hc in range(8):
                wt = q2w.tile([128, 8, 512], F32R)
                nc.sync.dma_start(out=wt[:], in_=fcw_d[hc])
                for ht in range(4):
                    pu = q2p.tile([128, TS], F32)
                    for c in range(8):
                        nc.tensor.matmul(
                            pu[:], wt[:, c, ht * 128:(ht + 1) * 128],
                            h2T[:, c, :], start=(c == 0), stop=(c == 7))
                    hi = hc * 4 + ht
                    usq = q2t.tile([128, TS], F32)
                    nc.scalar.activation(out=usq[:], in_=pu[:], func=AF.Square,
                                         bias=ab_sb[:, hi:hi + 1], scale=s_act)
                    nc.scalar.activation(out=actT[:, hi, :], in_=usq[:],
                                         func=AF.Exp, scale=-1.0)

        # ---- P3: c_fc2 + residual ----
        with tc.tile_pool(name="q3w", bufs=2) as q3w, \
             tc.tile_pool(name="q3o", bufs=3) as q3o, \
             tc.tile_pool(name="q3psum", bufs=1, space="PSUM") as q3p:
            po_tiles = []
            for tt in range(n_ttiles):
                row = []
                for co in range(2):
                    po_t = q3p.tile([128, 512], F32, tag=f"o{tt}{co}",
                                    name=f"po{tt}{co}")
                    row.append(po_t)
                po_tiles.append(row)
            for kr in range(8):
                w2 = q3w.tile([128, 4, C], F32R)
                nc.sync.dma_start(out=w2[:], in_=fc2w_d[kr])
                for tt in range(n_ttiles):
                    for k4 in range(4):
                        k = kr * 4 + k4
                        for co in range(2):
                            nc.tensor.matmul(
                                po_tiles[tt][co][:],
                                actT[:, k, tt * 128:(tt + 1) * 128],
                                w2[:, k4, co * 512:(co + 1) * 512],
                                start=(kr == 0 and k4 == 0),
                                stop=(kr == 7 and k4 == 3),
                                skip_group_check=True)
            for tt in range(n_ttiles):
                for co in range(2):
                    ot = q3o.tile([128, 512], F32)
                    nc.vector.tensor_add(ot[:], po_tiles[tt][co][:],
                                         x2pb[:, tt, co * 512:(co + 1) * 512])
                    nc.sync.dma_start(
                        out=out_d[tt * 128:(tt + 1) * 128,
                                  co * 512:(co + 1) * 512], in_=ot[:])
    nc.compile()
    return nc


# --------------------------------------------------------------------------
# Host-side orchestration
# --------------------------------------------------------------------------
_PROG_CACHE = {}


def _get_prog(key, builder, *args):
    if key not in _PROG_CACHE:
        _PROG_CACHE[key] = builder(*args)
    return _PROG_CACHE[key]


def _causal_masks4():
    s = np.arange(128)[:, None]
    t = np.arange(512)[None, :]
    ms = [((s + 128 * m) <= t).astype(np.float32) for m in range(4)]
    return np.ascontiguousarray(np.stack(ms, axis=1))  # [128, 4, 512]


def _perm(w, tiles, width):
    """[tiles*128, width] -> [128, tiles, width] (partition-major for DMA)."""
    return np.ascontiguousarray(w.reshape(tiles, 128, width).transpose(1, 0, 2))


def kernel(x, ln1_w, ln1_b, attn_w, attn_b, proj_w, proj_b,
           ln2_w, ln2_b, fc_w, fc_b, fc2_w, fc2_b,
           mu, sigma, gamma, beta, n_head):
    x = np.asarray(x, dtype=np.float32)
    attn_w = np.asarray(attn_w, dtype=np.float32)
    attn_b = np.asarray(attn_b, dtype=np.float32)
    proj_w = np.asarray(proj_w, dtype=np.float32)
    proj_b = np.asarray(proj_b, dtype=np.float32)
    fc_w = np.asarray(fc_w, dtype=np.float32)
    fc_b = np.asarray(fc_b, dtype=np.float32)
    fc2_w = np.asarray(fc2_w, dtype=np.float32)
    fc2_b = np.asarray(fc2_b, dtype=np.float32)
    ln1_w = np.asarray(ln1_w, dtype=np.float32)
    ln1_b = np.asarray(ln1_b, dtype=np.float32)
    ln2_w = np.asarray(ln2_w, dtype=np.float32)
    ln2_b = np.asarray(ln2_b, dtype=np.float32)
    mu = float(mu)
    sigma = float(sigma)
    gamma = float(gamma)
    beta = float(beta)
    n_head = int(n_head)

    B = x.shape[0]
    assert x.shape == (B, T, C) and B == 2 and n_head == 16

    _install_compile_cache()
    trace = bool(int(os.environ.get("BASS_KERNEL_TRACE", "0")))

    sig = abs(sigma) + 1e-8
    s_act = float(1.0 / (np.sqrt(2.0) * sig))

    # Fold LN affine params into the consuming projection weights (host-side).
    attn_w_eff = attn_w * ln1_w[None, :]
    attn_b_eff = attn_b + attn_w @ ln1_b
    fc_w_eff = fc_w * ln2_w[None, :]
    fc_b_eff = fc_b + fc_w @ ln2_b

    # ---- launch 1 ----
    nc1 = _get_prog(("l1",), build_l1)
    masks = _causal_masks4()
    vones = np.ones((128, 64), dtype=np.float32)
    ident = np.eye(128, dtype=np.float32)
    in_maps1 = []
    for c in range(N_CORES):
        b, hg = c // 4, c % 4
        q_rows = attn_w_eff[hg * 256:(hg + 1) * 256] * 0.125
        k_rows = attn_w_eff[C + hg * 256:C + (hg + 1) * 256]
        v_rows = attn_w_eff[2 * C + hg * 256:2 * C + (hg + 1) * 256]
        wqk = np.concatenate([q_rows, k_rows], axis=0)   # [512, 1024]
        bqk = np.concatenate([attn_b_eff[hg * 256:(hg + 1) * 256] * 0.125,
                              attn_b_eff[C + hg * 256:C + (hg + 1) * 256]])
        bv = attn_b_eff[2 * C + hg * 256:2 * C + (hg + 1) * 256]
        m = {
            "xT": np.ascontiguousarray(_perm(np.ascontiguousarray(x[b].T), 8, T).reshape(128, 8, 4, 512).transpose(2, 0, 1, 3)),
            "wqkT": _perm(np.ascontiguousarray(wqk.T), 8, 512),
            "wvT": _perm(np.ascontiguousarray(v_rows.T), 8, 256),
            "bqk": np.ascontiguousarray(bqk.reshape(4, 128).T),
            "bv": np.ascontiguousarray(bv[None, :]),
            "wsqk": np.ascontiguousarray(wqk.sum(axis=1).reshape(4, 128).T),
            "nwsv": np.ascontiguousarray(-v_rows.sum(axis=1)[None, :]),
            "projwT": _perm(
                np.ascontiguousarray(proj_w[:, hg * 256:(hg + 1) * 256].T), 2, 1024),
            "masks": masks,
            "vones": vones,
            "ident": ident,
        }
        in_maps1.append(m)
    res1 = run_bass_kernel_spmd(nc1, in_maps1, list(range(N_CORES)), trace=trace)
    if res1.exec_time_ns is not None:
        LAST_EXEC_NS["l1"] = res1.exec_time_ns
    yps = [res1.results[c]["yp"] for c in range(N_CORES)]

    # ---- launch 2 ----
    nc2 = _get_prog(("l2", s_act), build_l2, s_act)
    fc2w_eff = (gamma * fc2_w).T                        # [4096, 1024]
    fb2_eff = fc2_b + beta * fc2_w.sum(axis=1)
    abias = ((fc_b_eff - mu) * s_act).reshape(32, 128).T    # [128, 32]
    fcwT_p = _perm(np.ascontiguousarray(fc_w_eff.T), 8, HID)                      # [128, 8, 4096]
    fcw_chunks = np.ascontiguousarray(
        fcwT_p.reshape(128, 8, 8, 512).transpose(2, 0, 1, 3))   # [8,128,8,512]
    fc2wT_p = _perm(np.ascontiguousarray(fc2w_eff), 32, C)      # [128, 32, 1024]
    fc2w_chunks = np.ascontiguousarray(
        fc2wT_p.reshape(128, 8, 4, C).transpose(1, 0, 2, 3))    # [8,128,4,1024]
    ident = np.eye(128, dtype=np.float32)
    p = np.arange(128)
    gones = np.zeros((128, 4, 128), dtype=np.float32)
    for j in range(4):
        gones[p, j, 32 * j + (p % 32)] = 1.0
    in_maps2 = []
    for c in range(N_CORES):
        b, sl = c // 4, c % 4
        t0 = sl * TS
        yp4 = np.stack([yps[b * 4 + g][t0:t0 + TS] for g in range(4)])
        # interleave the 4 partials into 32-token groups: [16, 4*32, C]
        yp4s = np.ascontiguousarray(
            yp4.reshape(4, 16, 32, C).transpose(1, 0, 2, 3).reshape(16, 128, C))
        m = {
            "yp4": yp4s,
            "xs": np.ascontiguousarray(x[b, t0:t0 + TS]),
            "pb": proj_b[None, :],
            "fb2": np.ascontiguousarray(fb2_eff[None, :]),
            "abias": np.ascontiguousarray(abias),
            "gones": gones,
            "fcwT": fcw_chunks,
            "fc2wT": fc2w_chunks,
            "ident": ident,
        }
        in_maps2.append(m)
    res2 = run_bass_kernel_spmd(nc2, in_maps2, list(range(N_CORES)), trace=trace)
    if res2.exec_time_ns is not None:
        LAST_EXEC_NS["l2"] = res2.exec_time_ns

    out = np.empty((B, T, C), dtype=np.float32)
    for c in range(N_CORES):
        b, sl = c // 4, c % 4
        out[b, sl * TS:(sl + 1) * TS] = res2.results[c]["out"]
    return out



# revision 6
# speedup vs baseline: 1.2811x; 1.2811x over previous
"""Trainium2 Bass kernel for a dense transformer block (B=2, T=2048, C=1024, nh=16, H=4096).

Two SPMD launches over 8 NeuronCores (no device collectives):

  Launch 1 (head-parallel attention): cores 0-3 <- batch 0, cores 4-7 <- batch 1;
    each core does 4 heads over the full sequence. x arrives pre-transposed
    (feature-major) in bf16. LN1 statistics via ones-matmuls; the -mu*wsum and
    +bias LN terms are folded into the QKV PSUM accumulation as two rank-1
    matmuls, so the epilogue is a single VectorE multiply by the broadcast
    rstd. Q/K/V and attention run in bf16 (fp32 PSUM accumulate). The paired
    heads (even/odd) live in partition halves of the same feature tiles, so
    QK^T runs as 4 concurrent quadrant matmuls (tile_position packing, 2x).
    V carries a ones column so the softmax denominator falls out of the AV
    matmul; the divide happens right after each AV accumulation finishes.
    Output: un-projected attention y (feature-major, bf16) [128, 2, 2048].

  Host: pure re-slicing (no arithmetic on activations).

  Launch 2 (token-parallel): each core takes a 512-token slice: c_proj over
    the concatenated head outputs (contracts over all 1024 y-features, so no
    separate partial reduction is needed) + residual + proj_b -> LN2
    (bn_stats) -> transpose -> c_fc (bf16 weights) -> gaussian activation as a
    single Derivative_Erf pass (d/dx erf = 2/sqrt(pi) e^{-x^2}; sqrt(pi)/2,
    gamma, beta, mu, sigma and biases are folded host-side) -> c_fc2 (bf16)
    -> + residual -> output slice.
"""

import hashlib
import os
import shutil
from contextlib import ExitStack

import numpy as np
import ml_dtypes

import concourse.bass as bass
import concourse.tile as tile
from concourse import bacc, mybir
from concourse.bass_utils import run_bass_kernel_spmd

F32 = mybir.dt.float32
F32R = mybir.dt.float32r
BF16 = mybir.dt.bfloat16
AF = mybir.ActivationFunctionType
ALU = mybir.AluOpType
BF = ml_dtypes.bfloat16

N_CORES = 8
T = 2048          # tokens per batch
C = 1024          # model dim
NH_LOC = 4        # heads per core (launch 1)
HS = 64           # head size
HID = 4096        # mlp hidden
TS = 512          # tokens per core (launch 2)

LAST_EXEC_NS = {}  # launch name -> exec_time_ns (filled when tracing enabled)

_CACHE_DIR = "/tmp/neff_cache"


def _install_compile_cache():
    import concourse.bass2jax as b2j

    if getattr(b2j, "_neff_cache_installed", False):
        return
    real = b2j.compile_bir_kernel

    def cached(bir_json, tmpdir, neff_name="file.neff"):
        os.makedirs(_CACHE_DIR, exist_ok=True)
        h = hashlib.sha256(bir_json).hexdigest()
        cpath = os.path.join(_CACHE_DIR, h + ".neff")
        out = os.path.join(tmpdir, neff_name)
        if os.path.exists(cpath):
            shutil.copyfile(cpath, out)
            return out
        res = real(bir_json, tmpdir, neff_name)
        shutil.copyfile(res, cpath)
        return res

    b2j.compile_bir_kernel = cached
    b2j._neff_cache_installed = True


# --------------------------------------------------------------------------
# Launch 1: LN1 + QKV + causal attention (4 heads), un-projected y out
# --------------------------------------------------------------------------
def build_l1():
    nc = bacc.Bacc("TRN2", target_bir_lowering=False, debug=False,
                   num_devices=N_CORES)
    xT_d = nc.dram_tensor("xT", [4, 128, 8, 512], BF16, kind="ExternalInput")
    wqk_d = nc.dram_tensor("wqkT", [128, 8, 512], BF16, kind="ExternalInput")
    wv_d = nc.dram_tensor("wvT", [128, 8, 256], BF16, kind="ExternalInput")
    nws_d = nc.dram_tensor("nws", [1, 4, 128], F32R, kind="ExternalInput")
    bqk_d = nc.dram_tensor("bqk", [1, 4, 128], F32R, kind="ExternalInput")
    bv_d = nc.dram_tensor("bv", [1, 256], F32, kind="ExternalInput")
    nwsv_d = nc.dram_tensor("nwsv", [1, 256], F32, kind="ExternalInput")
    mask_d = nc.dram_tensor("masks", [128, 4, 512], BF16, kind="ExternalInput")
    vones_d = nc.dram_tensor("vones", [128, 1], BF16, kind="ExternalInput")
    id_d = nc.dram_tensor("ident", [128, 128], F32, kind="ExternalInput")
    yT_d = nc.dram_tensor("yT", [128, 2, T], BF16, kind="ExternalOutput")

    n_tc = T // 512              # 4 query chunks

    with tile.TileContext(nc) as tc, ExitStack() as ctx:
        consts = ctx.enter_context(tc.tile_pool(name="consts", bufs=1))
        vones_sb = consts.tile([128, 1], BF16)
        nc.sync.dma_start(out=vones_sb[:], in_=vones_d[:])
        eps_sb = consts.tile([128, 1], F32)
        nc.vector.memset(eps_sb[:], 1e-5)
        nws_sb = consts.tile([1, 4, 128], F32R)
        nc.sync.dma_start(out=nws_sb[:], in_=nws_d[:])
        bqk_sb = consts.tile([1, 4, 128], F32R)
        nc.sync.dma_start(out=bqk_sb[:], in_=bqk_d[:])
        ident = consts.tile([128, 128], F32)
        nc.sync.dma_start(out=ident[:], in_=id_d[:])
        bv_row = consts.tile([1, 256], F32)
        nc.sync.dma_start(out=bv_row[:], in_=bv_d[:])
        bv_b = consts.tile([128, 256], F32)
        nc.gpsimd.partition_broadcast(bv_b[:], bv_row[:])
        nwsv_row = consts.tile([1, 256], F32)
        nc.sync.dma_start(out=nwsv_row[:], in_=nwsv_d[:])
        nwsv_b = consts.tile([128, 256], F32)
        nc.gpsimd.partition_broadcast(nwsv_b[:], nwsv_row[:])

        big2 = ctx.enter_context(tc.tile_pool(name="big2", bufs=1))
        qkT = big2.tile([128, 4, T], BF16)     # Q feats (tiles 0,1), K feats (2,3)
        v_sb = big2.tile([128, 16, NH_LOC, 65], BF16)  # col 64 = ones
        yT = big2.tile([128, 2, T], BF16)
        nc.vector.memset(v_sb[:, :, :, 64:65], 1.0)

        wpool = ctx.enter_context(tc.tile_pool(name="wpool", bufs=1))
        wqk_sb = wpool.tile([128, 8, 512], BF16)
        nc.sync.dma_start(out=wqk_sb[:], in_=wqk_d[:])
        wv_sb = wpool.tile([128, 8, 256], BF16)
        nc.sync.dma_start(out=wv_sb[:], in_=wv_d[:])

        # ---- P1 per 512-token chunk: raw QKV matmuls on un-normalized xT;
        # LN folded in as rank-1 PSUM corrections + one epilogue multiply:
        #   qkv[f,t] = r[t]*(raw[f,t] - mu[t]*wsum[f] + sd[t]*b[f])
        with tc.tile_pool(name="p1", bufs=3) as p1, \
             tc.tile_pool(name="p1sq", bufs=3) as p1sq, \
             tc.tile_pool(name="p1sl", bufs=2) as p1sl, \
             tc.tile_pool(name="p1b", bufs=2) as p1b, \
             tc.tile_pool(name="p1c", bufs=2) as p1c, \
             tc.tile_pool(name="p1r", bufs=8) as p1r, \
             tc.tile_pool(name="p1psum", bufs=1, space="PSUM") as p1p, \
             tc.tile_pool(name="ptp", bufs=2, space="PSUM") as ptp, \
             tc.tile_pool(name="p2psum", bufs=2, space="PSUM") as p2p:
            for tch in range(n_tc):
                sl = slice(tch * 512, (tch + 1) * 512)
                slab = p1sl.tile([128, 8, 512], BF16, tag="slab")
                nc.sync.dma_start(out=slab[:], in_=xT_d[tch])
                ps_sum = p1p.tile([1, 512], F32, tag="s")
                ps_sq = p1p.tile([1, 512], F32, tag="q")
                for c in range(8):
                    sq = p1sq.tile([128, 512], BF16, tag="sq")
                    nc.vector.tensor_mul(sq[:], slab[:, c, :], slab[:, c, :])
                    nc.tensor.matmul(ps_sum[:], vones_sb[:], slab[:, c, :],
                                     start=(c == 0), stop=(c == 7),
                                     skip_group_check=True)
                    nc.tensor.matmul(ps_sq[:], vones_sb[:], sq[:],
                                     start=(c == 0), stop=(c == 7),
                                     skip_group_check=True)
                # row math: mu, sd, rstd (rows live on partition 0).
                # mrow/sdr are F32R because they feed matmuls as rhs.
                srow = p1r.tile([1, 512], F32, tag="row")
                nc.vector.tensor_copy(srow[:], ps_sum[:])
                qrow = p1r.tile([1, 512], F32, tag="row")
                nc.vector.tensor_copy(qrow[:], ps_sq[:])
                mrow = p1r.tile([1, 512], F32R, tag="rowr")
                nc.vector.tensor_scalar(out=mrow[:], in0=srow[:], scalar1=1.0 / C,
                                        scalar2=None, op0=ALU.mult)
                msq = p1r.tile([1, 512], F32, tag="row")
                nc.vector.tensor_mul(msq[:], mrow[:], mrow[:])
                vrow = p1r.tile([1, 512], F32, tag="row")
                nc.vector.scalar_tensor_tensor(out=vrow[:], in0=qrow[:],
                                               scalar=1.0 / C, in1=msq[:],
                                               op0=ALU.mult, op1=ALU.subtract)
                sdr = p1r.tile([1, 512], F32R, tag="rowr")
                nc.scalar.activation(out=sdr[:], in_=vrow[:], func=AF.Sqrt,
                                     bias=eps_sb[0:1], scale=1.0)
                rrow = p1r.tile([1, 512], F32, tag="row")
                nc.vector.reciprocal(rrow[:], sdr[:])
                rmurow = p1r.tile([1, 512], F32, tag="row")
                nc.vector.tensor_mul(rmurow[:], rrow[:], mrow[:])
                rb = p1b.tile([128, 512], F32, tag="rb")
                nc.gpsimd.partition_broadcast(rb[:], rrow[:])
                # token-major per-partition columns of r and r*mu (for V)
                rcol = p1c.tile([128, 4], F32, tag="rcol")
                rmucol = p1c.tile([128, 4], F32, tag="rmucol")
                for j in range(4):
                    tp1 = ptp.tile([128, 1], F32, tag="tp")
                    nc.tensor.transpose(tp1[:], rrow[0:1, j * 128:(j + 1) * 128],
                                        ident[0:1, 0:1])
                    nc.vector.tensor_copy(rcol[:, j:j + 1], tp1[:])
                    tp2 = ptp.tile([128, 1], F32, tag="tp")
                    nc.tensor.transpose(tp2[:], rmurow[0:1, j * 128:(j + 1) * 128],
                                        ident[0:1, 0:1])
                    nc.vector.tensor_copy(rmucol[:, j:j + 1], tp2[:])
                # raw Q/K projections + rank-1 LN corrections + epilogue
                for f in range(4):
                    ps = p2p.tile([128, 512], F32, tag="qk")
                    for c in range(8):
                        nc.tensor.matmul(
                            ps[:], wqk_sb[:, c, f * 128:(f + 1) * 128],
                            slab[:, c, :], start=(c == 0), stop=False,
                            skip_group_check=True)
                    nc.tensor.matmul(ps[:], nws_sb[:, f, :], mrow[:],
                                     start=False, stop=False,
                                     skip_group_check=True)
                    nc.tensor.matmul(ps[:], bqk_sb[:, f, :], sdr[:],
                                     start=False, stop=True,
                                     skip_group_check=True)
                    nc.vector.tensor_mul(qkT[:, f, sl], ps[:], rb[:])
                # raw V + LN epilogue (token-major)
                for tt4 in range(4):
                    tt = tch * 4 + tt4
                    ps = p2p.tile([128, 256], F32, tag="v")
                    for c in range(8):
                        nc.tensor.matmul(
                            ps[:], slab[:, c, tt4 * 128:(tt4 + 1) * 128],
                            wv_sb[:, c, :], start=(c == 0), stop=(c == 7))
                    corr = p1.tile([128, 256], F32, tag="corr")
                    nc.vector.scalar_tensor_tensor(
                        out=corr[:], in0=nwsv_b[:],
                        scalar=rmucol[:, tt4:tt4 + 1], in1=bv_b[:],
                        op0=ALU.mult, op1=ALU.add)
                    nc.vector.scalar_tensor_tensor(
                        out=v_sb[:, tt, :, 0:64],
                        in0=ps[:].rearrange("p (h d) -> p h d", h=NH_LOC),
                        scalar=rcol[:, tt4:tt4 + 1],
                        in1=corr[:].rearrange("p (h d) -> p h d", h=NH_LOC),
                        op0=ALU.mult, op1=ALU.add)

        # ---- P3: attention; paired heads (even/odd partition halves) share
        # quadrant-packed QK matmuls and one wide exp per (s, tcx).
        with tc.tile_pool(name="p3consts", bufs=1) as p3c, \
             tc.tile_pool(name="p3a", bufs=4) as p3a, \
             tc.tile_pool(name="p3y", bufs=4) as p3y, \
             tc.tile_pool(name="p3d", bufs=4) as p3d, \
             tc.tile_pool(name="p3sc", bufs=2, space="PSUM") as p3sc, \
             tc.tile_pool(name="p3py", bufs=1, space="PSUM") as p3py:
            mask_sb = p3c.tile([128, 4, 512], BF16)
            nc.sync.dma_start(out=mask_sb[:], in_=mask_d[:])
            for g in range(2):
                hA, hB = 2 * g, 2 * g + 1
                qf = g
                kf = 2 + g
                for pair in ((0, 1), (2, 3)):
                    smax = 4 * pair[1] + 4
                    pys = {}
                    for tcx in pair:
                        for h, po in ((hA, 0), (hB, 64)):
                            pys[(h, tcx)] = p3py.tile(
                                [65, 512], F32, tag=f"py{h % 2}{tcx % 2}",
                                name=f"py{h}_{tcx}")

                    def emit_qk(s):
                        tiles = []
                        for tcx in pair:
                            if s > 4 * tcx + 3:
                                continue
                            qsl = slice(tcx * 512, (tcx + 1) * 512)
                            sc = p3sc.tile([128, 1024], F32, tag="sc",
                                           name=f"sc{g}_{s}_{tcx}")
                            k0 = s * 128
                            for hj, po in ((0, 0), (1, 64)):
                                for kj in (0, 1):
                                    nc.tensor.matmul(
                                        sc[kj * 64:(kj + 1) * 64,
                                           hj * 512:(hj + 1) * 512],
                                        qkT[po:po + 64, kf,
                                            k0 + kj * 64:k0 + (kj + 1) * 64],
                                        qkT[po:po + 64, qf, qsl],
                                        start=True, stop=True,
                                        skip_group_check=True,
                                        tile_position=(po, kj * 64))
                            at = p3a.tile([128, 1024], BF16, tag="at",
                                          name=f"at{g}_{s}_{tcx}")
                            nc.scalar.activation(out=at[:], in_=sc[:],
                                                 func=AF.Exp)
                            if tcx == s // 4:
                                nc.vector.tensor_mul(
                                    at[:, 0:512], at[:, 0:512],
                                    mask_sb[:, s % 4, :])
                                nc.vector.tensor_mul(
                                    at[:, 512:1024], at[:, 512:1024],
                                    mask_sb[:, s % 4, :])
                            tiles.append((tcx, at))
                        return tiles

                    cur = emit_qk(0)
                    for s in range(smax):
                        nxt = emit_qk(s + 1) if s + 1 < smax else []
                        for tcx, at in cur:
                            qsl = slice(tcx * 512, (tcx + 1) * 512)
                            for h, po in ((hA, 0), (hB, 64)):
                                py = pys[(h, tcx)]
                                nc.tensor.matmul(
                                    py[:], v_sb[:, s, h, :],
                                    at[:, (po // 64) * 512:(po // 64) * 512 + 512],
                                    start=(s == 0), stop=(s == 4 * tcx + 3),
                                    skip_group_check=True)
                                if s == 4 * tcx + 3:
                                    yc = p3y.tile([65, 512], F32, tag="yc",
                                                  name=f"yc{h}_{tcx}")
                                    nc.vector.tensor_copy(yc[:], py[:])
                                    dn = p3d.tile([1, 512], F32, tag="dn",
                                                  name=f"dn{h}_{tcx}")
                                    nc.sync.dma_start(out=dn[:],
                                                      in_=yc[64:65, :])
                                    dnr = p3d.tile([1, 512], F32, tag="dnr",
                                                   name=f"dnr{h}_{tcx}")
                                    nc.vector.reciprocal(dnr[:], dn[:])
                                    db = p3d.tile([64, 512], F32, tag="db",
                                                  name=f"db{h}_{tcx}")
                                    nc.gpsimd.partition_broadcast(
                                        db[:], dnr[:])
                                    nc.vector.tensor_mul(
                                        yT[po:po + 64, g, qsl],
                                        yc[0:64, :], db[:])
                        cur = nxt
                    # stream finished token range of this head-pair out
                    psl = slice(pair[0] * 512, (pair[1] + 1) * 512)
                    nc.sync.dma_start(out=yT_d[:, g, psl], in_=yT[:, g, psl])
    nc.compile()
    return nc


# --------------------------------------------------------------------------
# Launch 2: c_proj + residual + LN2 + MLP + residual
# --------------------------------------------------------------------------
def build_l2(s_act: float):
    nc = bacc.Bacc("TRN2", target_bir_lowering=False, debug=False,
                   num_devices=N_CORES)
    yin_d = nc.dram_tensor("yin", [128, 8, 512], BF16, kind="ExternalInput")
    pw_d = nc.dram_tensor("pwT", [128, 8, 1024], BF16, kind="ExternalInput")
    xs_d = nc.dram_tensor("xs", [TS, C], F32, kind="ExternalInput")
    pb_d = nc.dram_tensor("pb", [1, C], F32, kind="ExternalInput")
    fb2_d = nc.dram_tensor("fb2", [1, C], F32, kind="ExternalInput")
    ab_d = nc.dram_tensor("abias", [128, 32], F32, kind="ExternalInput")
    fcw_d = nc.dram_tensor("fcwT", [8, 128, 8, 512], BF16, kind="ExternalInput")
    fc2w_d = nc.dram_tensor("fc2wT", [8, 128, 4, C], BF16, kind="ExternalInput")
    idb_d = nc.dram_tensor("identb", [128, 128], BF16, kind="ExternalInput")
    out_d = nc.dram_tensor("out", [TS, C], F32, kind="ExternalOutput")

    n_ttiles = TS // 128    # 4

    with tile.TileContext(nc) as tc, ExitStack() as ctx:
        consts = ctx.enter_context(tc.tile_pool(name="consts", bufs=1))
        identb = consts.tile([128, 128], BF16)
        nc.sync.dma_start(out=identb[:], in_=idb_d[:])
        eps_sb = consts.tile([128, 1], F32)
        nc.vector.memset(eps_sb[:], 1e-5)
        pb_row = consts.tile([1, C], F32)
        nc.sync.dma_start(out=pb_row[:], in_=pb_d[:])
        pb_b = consts.tile([128, C], F32)
        nc.gpsimd.partition_broadcast(pb_b[:], pb_row[:])
        fb2_row = consts.tile([1, C], F32)
        nc.sync.dma_start(out=fb2_row[:], in_=fb2_d[:])
        fb2_b = consts.tile([128, C], F32)
        nc.gpsimd.partition_broadcast(fb2_b[:], fb2_row[:])
        ab_sb = consts.tile([128, 32], F32)
        nc.sync.dma_start(out=ab_sb[:], in_=ab_d[:])

        big = ctx.enter_context(tc.tile_pool(name="big", bufs=1))
        h2T = big.tile([128, 8, TS], BF16)         # 8KB/p
        x2pb = big.tile([128, n_ttiles, C], F32)   # x2 + fc2 bias, 16KB/p
        actT = big.tile([128, 32, TS], BF16)       # 32KB/p
        yin_sb = big.tile([128, 8, 512], BF16)
        nc.sync.dma_start(out=yin_sb[:], in_=yin_d[:])
        pw_sb = big.tile([128, 8, 1024], BF16)
        nc.sync.dma_start(out=pw_sb[:], in_=pw_d[:])

        # ---- P1: c_proj + residual + proj bias, LN2, transpose ----
        with tc.tile_pool(name="q1", bufs=3) as q1, \
             tc.tile_pool(name="q1s", bufs=4) as q1s, \
             tc.tile_pool(name="q1psum", bufs=2, space="PSUM") as q1p, \
             tc.tile_pool(name="q1pt", bufs=2, space="PSUM") as q1pt:
            for tt in range(n_ttiles):
                x2 = q1.tile([128, C], F32, tag="x2")
                px2 = [q1p.tile([128, 512], F32, tag=f"px{ch}", name=f"px{tt}_{ch}")
                       for ch in range(2)]
                for k in range(8):
                    for ch in range(2):
                        nc.tensor.matmul(
                            px2[ch][:], yin_sb[:, k, tt * 128:(tt + 1) * 128],
                            pw_sb[:, k, ch * 512:(ch + 1) * 512],
                            start=(k == 0), stop=(k == 7),
                            skip_group_check=True)
                xst = q1.tile([128, C], F32, tag="xs")
                nc.sync.dma_start(out=xst[:], in_=xs_d[tt * 128:(tt + 1) * 128, :])
                xpb = q1.tile([128, C], F32, tag="xpb")
                nc.gpsimd.tensor_add(xpb[:], xst[:], pb_b[:])
                for ch in range(2):
                    csl = slice(ch * 512, (ch + 1) * 512)
                    nc.vector.tensor_add(x2[:, csl], px2[ch][:], xpb[:, csl])
                nc.vector.tensor_add(x2pb[:, tt, :], x2[:], fb2_b[:])
                stats = q1s.tile([128, 2, 6], F32)
                x2g = x2[:].rearrange("p (g d) -> p g d", g=2)
                nc.vector.bn_stats(out=stats[:, 0, :], in_=x2g[:, 0, :])
                nc.vector.bn_stats(out=stats[:, 1, :], in_=x2g[:, 1, :])
                mv = q1s.tile([128, 2], F32)
                nc.vector.bn_aggr(out=mv[:], in_=stats[:])
                sd = q1s.tile([128, 1], F32, tag="sd")
                nc.scalar.activation(out=sd[:], in_=mv[:, 1:2], func=AF.Sqrt,
                                     bias=eps_sb[:], scale=1.0)
                rstd = q1s.tile([128, 1], F32)
                nc.vector.reciprocal(rstd[:], sd[:])
                h2 = q1.tile([128, C], BF16, tag="h2")
                nc.vector.tensor_scalar(out=h2[:], in0=x2[:],
                                        scalar1=mv[:, 0:1], scalar2=rstd[:],
                                        op0=ALU.subtract, op1=ALU.mult)
                for c in range(8):
                    pt = q1pt.tile([128, 128], BF16)
                    nc.tensor.transpose(pt[:], h2[:, c * 128:(c + 1) * 128],
                                        identb[:])
                    nc.vector.tensor_copy(h2T[:, c, tt * 128:(tt + 1) * 128],
                                          pt[:])

        # ---- P2: c_fc + gaussian activation via Derivative_Erf ----
        with tc.tile_pool(name="q2w", bufs=2) as q2w, \
             tc.tile_pool(name="q2psum", bufs=3, space="PSUM") as q2p:
            for hc in range(8):
                wt = q2w.tile([128, 8, 512], BF16)
                nc.sync.dma_start(out=wt[:], in_=fcw_d[hc])
                for ht in range(4):
                    pu = q2p.tile([128, TS], F32)
                    for c in range(8):
                        nc.tensor.matmul(
                            pu[:], wt[:, c, ht * 128:(ht + 1) * 128],
                            h2T[:, c, :], start=(c == 0), stop=(c == 7))
                    hi = hc * 4 + ht
                    nc.scalar.activation(out=actT[:, hi, :], in_=pu[:],
                                         func=AF.Derivative_Erf,
                                         bias=ab_sb[:, hi:hi + 1], scale=s_act)

        # ---- P3: c_fc2 + residual ----
        with tc.tile_pool(name="q3w", bufs=2) as q3w, \
             tc.tile_pool(name="q3o", bufs=3) as q3o, \
             tc.tile_pool(name="q3psum", bufs=1, space="PSUM") as q3p:
            po_tiles = []
            for tt in range(n_ttiles):
                row = []
                for co in range(2):
                    po_t = q3p.tile([128, 512], F32, tag=f"o{tt}{co}",
                                    name=f"po{tt}{co}")
                    row.append(po_t)
                po_tiles.append(row)
            for kr in range(8):
                w2 = q3w.tile([128, 4, C], BF16)
                nc.sync.dma_start(out=w2[:], in_=fc2w_d[kr])
                for tt in range(n_ttiles):
                    for k4 in range(4):
                        k = kr * 4 + k4
                        for co in range(2):
                            nc.tensor.matmul(
                                po_tiles[tt][co][:],
                                actT[:, k, tt * 128:(tt + 1) * 128],
                                w2[:, k4, co * 512:(co + 1) * 512],
                                start=(kr == 0 and k4 == 0),
                                stop=(kr == 7 and k4 == 3),
                                skip_group_check=True)
            for tt in range(n_ttiles):
                for co in range(2):
                    ot = q3o.tile([128, 512], F32)
                    nc.vector.tensor_add(ot[:], po_tiles[tt][co][:],
                                         x2pb[:, tt, co * 512:(co + 1) * 512])
                    nc.sync.dma_start(
                        out=out_d[tt * 128:(tt + 1) * 128,
                                  co * 512:(co + 1) * 512], in_=ot[:])
    nc.compile()
    return nc


# --------------------------------------------------------------------------
# Host-side orchestration
# --------------------------------------------------------------------------
_PROG_CACHE = {}


def _get_prog(key, builder, *args):
    if key not in _PROG_CACHE:
        _PROG_CACHE[key] = builder(*args)
    return _PROG_CACHE[key]


def _causal_masks4():
    s = np.arange(128)[:, None]
    t = np.arange(512)[None, :]
    ms = [((s + 128 * m) <= t).astype(np.float32) for m in range(4)]
    return np.ascontiguousarray(np.stack(ms, axis=1))  # [128, 4, 512]


def _perm(w, tiles, width):
    """[tiles*128, width] -> [128, tiles, width] (partition-major for DMA)."""
    return np.ascontiguousarray(w.reshape(tiles, 128, width).transpose(1, 0, 2))


def _bf(a):
    return np.ascontiguousarray(np.asarray(a, dtype=np.float32).astype(BF))


def kernel(x, ln1_w, ln1_b, attn_w, attn_b, proj_w, proj_b,
           ln2_w, ln2_b, fc_w, fc_b, fc2_w, fc2_b,
           mu, sigma, gamma, beta, n_head):
    x = np.asarray(x, dtype=np.float32)
    attn_w = np.asarray(attn_w, dtype=np.float32)
    attn_b = np.asarray(attn_b, dtype=np.float32)
    proj_w = np.asarray(proj_w, dtype=np.float32)
    proj_b = np.asarray(proj_b, dtype=np.float32)
    fc_w = np.asarray(fc_w, dtype=np.float32)
    fc_b = np.asarray(fc_b, dtype=np.float32)
    fc2_w = np.asarray(fc2_w, dtype=np.float32)
    fc2_b = np.asarray(fc2_b, dtype=np.float32)
    ln1_w = np.asarray(ln1_w, dtype=np.float32)
    ln1_b = np.asarray(ln1_b, dtype=np.float32)
    ln2_w = np.asarray(ln2_w, dtype=np.float32)
    ln2_b = np.asarray(ln2_b, dtype=np.float32)
    mu = float(mu)
    sigma = float(sigma)
    gamma = float(gamma)
    beta = float(beta)
    n_head = int(n_head)

    B = x.shape[0]
    assert x.shape == (B, T, C) and B == 2 and n_head == 16

    _install_compile_cache()
    trace = bool(int(os.environ.get("BASS_KERNEL_TRACE", "0")))

    sig = abs(sigma) + 1e-8
    s_act = float(1.0 / (np.sqrt(2.0) * sig))

    # Fold LN affine params into the consuming projection weights (host-side).
    attn_w_eff = attn_w * ln1_w[None, :]
    attn_b_eff = attn_b + attn_w @ ln1_b
    fc_w_eff = fc_w * ln2_w[None, :]
    fc_b_eff = fc_b + fc_w @ ln2_b

    # ---- launch 1 ----
    nc1 = _get_prog(("l1",), build_l1)
    masks = _bf(_causal_masks4())
    vones = _bf(np.ones((128, 1), dtype=np.float32))
    ident = np.eye(128, dtype=np.float32)
    in_maps1 = []
    for c in range(N_CORES):
        b, hg = c // 4, c % 4
        q_rows = attn_w_eff[hg * 256:(hg + 1) * 256] * 0.125
        k_rows = attn_w_eff[C + hg * 256:C + (hg + 1) * 256]
        v_rows = attn_w_eff[2 * C + hg * 256:2 * C + (hg + 1) * 256]
        wqk = np.concatenate([q_rows, k_rows], axis=0)   # [512, 1024]
        bqk = np.concatenate([attn_b_eff[hg * 256:(hg + 1) * 256] * 0.125,
                              attn_b_eff[C + hg * 256:C + (hg + 1) * 256]])
        bv = attn_b_eff[2 * C + hg * 256:2 * C + (hg + 1) * 256]
        m = {
            "xT": _bf(_perm(np.ascontiguousarray(x[b].T), 8, T)
                      .reshape(128, 8, 4, 512).transpose(2, 0, 1, 3)),
            "wqkT": _bf(_perm(np.ascontiguousarray(wqk.T), 8, 512)),
            "wvT": _bf(_perm(np.ascontiguousarray(v_rows.T), 8, 256)),
            "nws": np.ascontiguousarray(-wqk.sum(axis=1).reshape(1, 4, 128)),
            "bqk": np.ascontiguousarray(bqk.reshape(1, 4, 128)),
            "bv": np.ascontiguousarray(bv[None, :]),
            "nwsv": np.ascontiguousarray(-v_rows.sum(axis=1)[None, :]),
            "masks": masks,
            "vones": vones,
            "ident": ident,
        }
        in_maps1.append(m)
    res1 = run_bass_kernel_spmd(nc1, in_maps1, list(range(N_CORES)), trace=trace)
    if res1.exec_time_ns is not None:
        LAST_EXEC_NS["l1"] = res1.exec_time_ns
    # yT [128, 2, 2048]: head h local = 2*cl + (po//64); reorder to [256, 2048]
    ystrips = []
    for c in range(N_CORES):
        yt = np.asarray(res1.results[c]["yT"])
        ystrips.append(np.ascontiguousarray(
            yt.reshape(2, 64, 2, T).transpose(2, 0, 1, 3).reshape(256, T)))

    # ---- launch 2 ----
    nc2 = _get_prog(("l2", s_act), build_l2, s_act)
    fc2w_eff = (gamma * np.sqrt(np.pi) / 2.0 * fc2_w).T      # [4096, 1024]
    fb2_eff = fc2_b + beta * fc2_w.sum(axis=1)
    abias = ((fc_b_eff - mu) * s_act).reshape(32, 128).T     # [128, 32]
    fcwT_p = _perm(np.ascontiguousarray(fc_w_eff.T), 8, HID)      # [128,8,4096]
    fcw_chunks = _bf(
        fcwT_p.reshape(128, 8, 8, 512).transpose(2, 0, 1, 3))     # [8,128,8,512]
    fc2wT_p = _perm(np.ascontiguousarray(fc2w_eff), 32, C)        # [128,32,1024]
    fc2w_chunks = _bf(
        fc2wT_p.reshape(128, 8, 4, C).transpose(1, 0, 2, 3))      # [8,128,4,1024]
    pwT = _bf(_perm(np.ascontiguousarray(proj_w.T), 8, C))        # [128,8,1024]
    identb = _bf(np.eye(128, dtype=np.float32))
    in_maps2 = []
    for c in range(N_CORES):
        b, slc = c // 4, c % 4
        t0 = slc * TS
        yin = np.concatenate(
            [ystrips[b * 4 + g][:, t0:t0 + TS] for g in range(4)], axis=0)
        m = {
            "yin": np.ascontiguousarray(
                yin.reshape(8, 128, TS).transpose(1, 0, 2)),
            "pwT": pwT,
            "xs": np.ascontiguousarray(x[b, t0:t0 + TS]),
            "pb": proj_b[None, :],
            "fb2": np.ascontiguousarray(fb2_eff[None, :]),
            "abias": np.ascontiguousarray(abias),
            "fcwT": fcw_chunks,
            "fc2wT": fc2w_chunks,
            "identb": identb,
        }
        in_maps2.append(m)
    res2 = run_bass_kernel_spmd(nc2, in_maps2, list(range(N_CORES)), trace=trace)
    if res2.exec_time_ns is not None:
        LAST_EXEC_NS["l2"] = res2.exec_time_ns

    out = np.empty((B, T, C), dtype=np.float32)
    for c in range(N_CORES):
        b, slc = c // 4, c % 4
        out[b, slc * TS:(slc + 1) * TS] = res2.results[c]["out"]
    return out


# revision 11
# speedup vs baseline: 1.3870x; 1.0826x over previous
"""Trainium2 Bass kernel for a dense transformer block (B=2, T=2048, C=1024, nh=16, H=4096).

Two SPMD launches over 8 NeuronCores (no device collectives):

  Launch 1 (head-parallel attention): cores 0-3 <- batch 0, cores 4-7 <- batch 1;
    each core does 4 heads over the full sequence. x arrives pre-transposed
    (feature-major) in bf16. LN1 statistics via ones-matmuls; the -mu*wsum and
    +bias LN terms are folded into the QKV PSUM accumulation as two rank-1
    matmuls, so the epilogue is a single VectorE multiply by the broadcast
    rstd. Q/K/V and attention run in bf16 (fp32 PSUM accumulate). The paired
    heads (even/odd) live in partition halves of the same feature tiles, so
    QK^T runs as 4 concurrent quadrant matmuls (tile_position packing, 2x).
    V carries a ones column so the softmax denominator falls out of the AV
    matmul; the divide happens right after each AV accumulation finishes.
    Output: un-projected attention y (feature-major, bf16) [128, 2, 2048].

  Host: pure re-slicing (no arithmetic on activations).

  Launch 2 (token-parallel): each core takes a 512-token slice: c_proj over
    the concatenated head outputs (contracts over all 1024 y-features, so no
    separate partial reduction is needed) + residual + proj_b -> LN2
    (bn_stats) -> transpose -> c_fc (bf16 weights) -> gaussian activation as a
    single Derivative_Erf pass (d/dx erf = 2/sqrt(pi) e^{-x^2}; sqrt(pi)/2,
    gamma, beta, mu, sigma and biases are folded host-side) -> c_fc2 (bf16)
    -> + residual -> output slice.
"""

import hashlib
import os
import shutil
from contextlib import ExitStack

import numpy as np
import ml_dtypes

import concourse.bass as bass
import concourse.tile as tile
from concourse import bacc, mybir
from concourse.bass_utils import run_bass_kernel_spmd

F32 = mybir.dt.float32
F32R = mybir.dt.float32r
BF16 = mybir.dt.bfloat16
AF = mybir.ActivationFunctionType
ALU = mybir.AluOpType
BF = ml_dtypes.bfloat16

N_CORES = 8
T = 2048          # tokens per batch
C = 1024          # model dim
NH_LOC = 4        # heads per core (launch 1)
HS = 64           # head size
HID = 4096        # mlp hidden
TS = 512          # tokens per core (launch 2)

LAST_EXEC_NS = {}  # launch name -> exec_time_ns (filled when tracing enabled)

_CACHE_DIR = "/tmp/neff_cache"


def _install_compile_cache():
    import concourse.bass2jax as b2j

    if getattr(b2j, "_neff_cache_installed", False):
        return
    real = b2j.compile_bir_kernel

    def cached(bir_json, tmpdir, neff_name="file.neff"):
        os.makedirs(_CACHE_DIR, exist_ok=True)
        h = hashlib.sha256(bir_json).hexdigest()
        cpath = os.path.join(_CACHE_DIR, h + ".neff")
        out = os.path.join(tmpdir, neff_name)
        if os.path.exists(cpath):
            shutil.copyfile(cpath, out)
            return out
        res = real(bir_json, tmpdir, neff_name)
        shutil.copyfile(res, cpath)
        return res

    b2j.compile_bir_kernel = cached
    b2j._neff_cache_installed = True


# --------------------------------------------------------------------------
# Launch 1: LN1 + QKV + causal attention (4 heads), un-projected y out
# --------------------------------------------------------------------------
def build_l1():
    nc = bacc.Bacc("TRN2", target_bir_lowering=False, debug=False,
                   num_devices=N_CORES)
    xT_d = nc.dram_tensor("xT", [4, 128, 8, 512], BF16, kind="ExternalInput")
    wqk_d = nc.dram_tensor("wqkT", [128, 8, 512], BF16, kind="ExternalInput")
    wv_d = nc.dram_tensor("wvT", [128, 8, 256], BF16, kind="ExternalInput")
    nws_d = nc.dram_tensor("nws", [1, 4, 128], F32R, kind="ExternalInput")
    bqk_d = nc.dram_tensor("bqk", [1, 4, 128], F32R, kind="ExternalInput")
    bv_d = nc.dram_tensor("bv", [1, 256], F32, kind="ExternalInput")
    nwsv_d = nc.dram_tensor("nwsv", [1, 256], F32, kind="ExternalInput")
    mask_d = nc.dram_tensor("masks", [128, 4, 512], BF16, kind="ExternalInput")
    vones_d = nc.dram_tensor("vones", [128, 1], BF16, kind="ExternalInput")
    id_d = nc.dram_tensor("ident", [128, 128], F32, kind="ExternalInput")
    yT_d = nc.dram_tensor("yT", [128, 2, T], BF16, kind="ExternalOutput")

    n_tc = T // 512              # 4 query chunks

    with tile.TileContext(nc) as tc, ExitStack() as ctx:
        consts = ctx.enter_context(tc.tile_pool(name="consts", bufs=1))
        vones_sb = consts.tile([128, 1], BF16)
        nc.sync.dma_start(out=vones_sb[:], in_=vones_d[:])
        eps_sb = consts.tile([128, 1], F32)
        nc.vector.memset(eps_sb[:], 1e-5)
        nws_sb = consts.tile([1, 4, 128], F32R)
        nc.sync.dma_start(out=nws_sb[:], in_=nws_d[:])
        bqk_sb = consts.tile([1, 4, 128], F32R)
        nc.sync.dma_start(out=bqk_sb[:], in_=bqk_d[:])
        ident = consts.tile([128, 128], F32)
        nc.sync.dma_start(out=ident[:], in_=id_d[:])
        bv_row = consts.tile([1, 256], F32)
        nc.sync.dma_start(out=bv_row[:], in_=bv_d[:])
        bv_b = consts.tile([128, 256], F32)
        nc.gpsimd.partition_broadcast(bv_b[:], bv_row[:])
        nwsv_row = consts.tile([1, 256], F32)
        nc.sync.dma_start(out=nwsv_row[:], in_=nwsv_d[:])
        nwsv_b = consts.tile([128, 256], F32)
        nc.gpsimd.partition_broadcast(nwsv_b[:], nwsv_row[:])

        big2 = ctx.enter_context(tc.tile_pool(name="big2", bufs=1))
        qkT = big2.tile([128, 4, T], BF16)     # Q feats (tiles 0,1), K feats (2,3)
        v_sb = big2.tile([128, 16, NH_LOC, 65], BF16)  # col 64 = ones
        yT = big2.tile([128, 2, T], BF16)
        nc.vector.memset(v_sb[:, :, :, 64:65], 1.0)

        wpool = ctx.enter_context(tc.tile_pool(name="wpool", bufs=1))
        wqk_sb = wpool.tile([128, 8, 512], BF16)
        nc.sync.dma_start(out=wqk_sb[:], in_=wqk_d[:])
        wv_sb = wpool.tile([128, 8, 256], BF16)
        nc.sync.dma_start(out=wv_sb[:], in_=wv_d[:])

        # ---- P1 per 512-token chunk: raw QKV matmuls on un-normalized xT;
        # LN folded in as rank-1 PSUM corrections + one epilogue multiply:
        #   qkv[f,t] = r[t]*(raw[f,t] - mu[t]*wsum[f] + sd[t]*b[f])
        with tc.tile_pool(name="p1", bufs=3) as p1, \
             tc.tile_pool(name="p1sq", bufs=3) as p1sq, \
             tc.tile_pool(name="p1sl", bufs=2) as p1sl, \
             tc.tile_pool(name="p1b", bufs=2) as p1b, \
             tc.tile_pool(name="p1c", bufs=2) as p1c, \
             tc.tile_pool(name="p1r", bufs=8) as p1r, \
             tc.tile_pool(name="p1psum", bufs=1, space="PSUM") as p1p, \
             tc.tile_pool(name="ptp", bufs=2, space="PSUM") as ptp, \
             tc.tile_pool(name="p2psum", bufs=2, space="PSUM") as p2p:
            for tch in range(n_tc):
                sl = slice(tch * 512, (tch + 1) * 512)
                slab = p1sl.tile([128, 8, 512], BF16, tag="slab")
                nc.sync.dma_start(out=slab[:], in_=xT_d[tch])
                ps_sum = p1p.tile([1, 512], F32, tag="s")
                ps_sq = p1p.tile([1, 512], F32, tag="q")
                for c in range(8):
                    sq = p1sq.tile([128, 512], BF16, tag="sq")
                    nc.vector.tensor_mul(sq[:], slab[:, c, :], slab[:, c, :])
                    nc.tensor.matmul(ps_sum[:], vones_sb[:], slab[:, c, :],
                                     start=(c == 0), stop=(c == 7),
                                     skip_group_check=True)
                    nc.tensor.matmul(ps_sq[:], vones_sb[:], sq[:],
                                     start=(c == 0), stop=(c == 7),
                                     skip_group_check=True)
                # row math: mu, sd, rstd (rows live on partition 0).
                # mrow/sdr are F32R because they feed matmuls as rhs.
                srow = p1r.tile([1, 512], F32, tag="row")
                nc.vector.tensor_copy(srow[:], ps_sum[:])
                qrow = p1r.tile([1, 512], F32, tag="row")
                nc.vector.tensor_copy(qrow[:], ps_sq[:])
                mrow = p1r.tile([1, 512], F32R, tag="rowr")
                nc.vector.tensor_scalar(out=mrow[:], in0=srow[:], scalar1=1.0 / C,
                                        scalar2=None, op0=ALU.mult)
                msq = p1r.tile([1, 512], F32, tag="row")
                nc.vector.tensor_mul(msq[:], mrow[:], mrow[:])
                vrow = p1r.tile([1, 512], F32, tag="row")
                nc.vector.scalar_tensor_tensor(out=vrow[:], in0=qrow[:],
                                               scalar=1.0 / C, in1=msq[:],
                                               op0=ALU.mult, op1=ALU.subtract)
                sdr = p1r.tile([1, 512], F32R, tag="rowr")
                nc.scalar.activation(out=sdr[:], in_=vrow[:], func=AF.Sqrt,
                                     bias=eps_sb[0:1], scale=1.0)
                rrow = p1r.tile([1, 512], F32, tag="row")
                nc.vector.reciprocal_approx_fast(rrow[:],
                                                 sdr[:].bitcast(F32))
                rmurow = p1r.tile([1, 512], F32, tag="row")
                nc.vector.tensor_mul(rmurow[:], rrow[:], mrow[:])
                rb = p1b.tile([128, 512], F32, tag="rb")
                nc.gpsimd.partition_broadcast(rb[:], rrow[:])
                # token-major per-partition columns of r and r*mu (for V)
                rcol = p1c.tile([128, 4], F32, tag="rcol")
                rmucol = p1c.tile([128, 4], F32, tag="rmucol")
                for j in range(4):
                    tp1 = ptp.tile([128, 1], F32, tag="tp")
                    nc.tensor.transpose(tp1[:], rrow[0:1, j * 128:(j + 1) * 128],
                                        ident[0:1, 0:1])
                    nc.vector.tensor_copy(rcol[:, j:j + 1], tp1[:])
                    tp2 = ptp.tile([128, 1], F32, tag="tp")
                    nc.tensor.transpose(tp2[:], rmurow[0:1, j * 128:(j + 1) * 128],
                                        ident[0:1, 0:1])
                    nc.vector.tensor_copy(rmucol[:, j:j + 1], tp2[:])
                # raw Q/K projections + rank-1 LN corrections + epilogue
                for f in range(4):
                    ps = p2p.tile([128, 512], F32, tag="qk")
                    for c in range(8):
                        nc.tensor.matmul(
                            ps[:], wqk_sb[:, c, f * 128:(f + 1) * 128],
                            slab[:, c, :], start=(c == 0), stop=False,
                            skip_group_check=True)
                    nc.tensor.matmul(ps[:], nws_sb[:, f, :], mrow[:],
                                     start=False, stop=False,
                                     skip_group_check=True)
                    nc.tensor.matmul(ps[:], bqk_sb[:, f, :], sdr[:],
                                     start=False, stop=True,
                                     skip_group_check=True)
                    nc.vector.tensor_mul(qkT[:, f, sl], ps[:], rb[:])
                # raw V + LN epilogue (token-major)
                for tt4 in range(4):
                    tt = tch * 4 + tt4
                    ps = p2p.tile([128, 256], F32, tag="v")
                    for c in range(8):
                        nc.tensor.matmul(
                            ps[:], slab[:, c, tt4 * 128:(tt4 + 1) * 128],
                            wv_sb[:, c, :], start=(c == 0), stop=(c == 7))
                    corr = p1.tile([128, 256], F32, tag="corr")
                    nc.vector.scalar_tensor_tensor(
                        out=corr[:], in0=nwsv_b[:],
                        scalar=rmucol[:, tt4:tt4 + 1], in1=bv_b[:],
                        op0=ALU.mult, op1=ALU.add)
                    nc.vector.scalar_tensor_tensor(
                        out=v_sb[:, tt, :, 0:64],
                        in0=ps[:].rearrange("p (h d) -> p h d", h=NH_LOC),
                        scalar=rcol[:, tt4:tt4 + 1],
                        in1=corr[:].rearrange("p (h d) -> p h d", h=NH_LOC),
                        op0=ALU.mult, op1=ALU.add)

        # ---- P3: attention; paired heads (even/odd partition halves) share
        # quadrant-packed QK matmuls and one wide exp per (s, tcx).
        with tc.tile_pool(name="p3consts", bufs=1) as p3c, \
             tc.tile_pool(name="p3a", bufs=4) as p3a, \
             tc.tile_pool(name="p3y", bufs=4) as p3y, \
             tc.tile_pool(name="p3d", bufs=4) as p3d, \
             tc.tile_pool(name="p3sc", bufs=2, space="PSUM") as p3sc, \
             tc.tile_pool(name="p3py", bufs=1, space="PSUM") as p3py:
            mask_sb = p3c.tile([128, 4, 512], BF16)
            nc.sync.dma_start(out=mask_sb[:], in_=mask_d[:])
            for g in range(2):
                hA, hB = 2 * g, 2 * g + 1
                qf = g
                kf = 2 + g
                for pair in ((0, 1), (2, 3)):
                    smax = 4 * pair[1] + 4
                    pys = {}
                    for tcx in pair:
                        for h, po in ((hA, 0), (hB, 64)):
                            pys[(h, tcx)] = p3py.tile(
                                [65, 512], F32, tag=f"py{h % 2}{tcx % 2}",
                                name=f"py{h}_{tcx}")

                    def emit_qk(s):
                        tiles = []
                        for tcx in pair:
                            if s > 4 * tcx + 3:
                                continue
                            qsl = slice(tcx * 512, (tcx + 1) * 512)
                            sc = p3sc.tile([128, 1024], F32, tag="sc",
                                           name=f"sc{g}_{s}_{tcx}")
                            k0 = s * 128
                            for hj, po in ((0, 0), (1, 64)):
                                for kj in (0, 1):
                                    nc.tensor.matmul(
                                        sc[kj * 64:(kj + 1) * 64,
                                           hj * 512:(hj + 1) * 512],
                                        qkT[po:po + 64, kf,
                                            k0 + kj * 64:k0 + (kj + 1) * 64],
                                        qkT[po:po + 64, qf, qsl],
                                        start=True, stop=True,
                                        skip_group_check=True,
                                        tile_position=(po, kj * 64))
                            at = p3a.tile([128, 1024], BF16, tag="at",
                                          name=f"at{g}_{s}_{tcx}")
                            nc.scalar.activation(out=at[:], in_=sc[:],
                                                 func=AF.Exp)
                            if tcx == s // 4:
                                atm = p3a.tile([128, 1024], BF16, tag="atm",
                                               name=f"atm{g}_{s}_{tcx}")
                                nc.vector.tensor_mul(atm[:, 0:512],
                                                     at[:, 0:512],
                                                     mask_sb[:, s % 4, :])
                                nc.vector.tensor_mul(atm[:, 512:1024],
                                                     at[:, 512:1024],
                                                     mask_sb[:, s % 4, :])
                                at = atm
                            tiles.append((tcx, at))
                        return tiles

                    cur = emit_qk(0)
                    for s in range(smax):
                        nxt = emit_qk(s + 1) if s + 1 < smax else []
                        for tcx, at in cur:
                            qsl = slice(tcx * 512, (tcx + 1) * 512)
                            for h, po in ((hA, 0), (hB, 64)):
                                py = pys[(h, tcx)]
                                nc.tensor.matmul(
                                    py[:], v_sb[:, s, h, :],
                                    at[:, (po // 64) * 512:(po // 64) * 512 + 512],
                                    start=(s == 0), stop=(s == 4 * tcx + 3),
                                    skip_group_check=True)
                                if s == 4 * tcx + 3:
                                    dcp = p3y.tile([65, 512], F32, tag="dcp",
                                                   name=f"dcp{h}_{tcx}")
                                    nc.vector.tensor_copy(dcp[64:65, :],
                                                          py[64:65, :])
                                    dn = p3d.tile([1, 512], F32, tag="dn",
                                                  name=f"dn{h}_{tcx}")
                                    nc.sync.dma_start(out=dn[:],
                                                      in_=dcp[64:65, :])
                                    dnr = p3d.tile([1, 512], F32, tag="dnr",
                                                   name=f"dnr{h}_{tcx}")
                                    nc.vector.reciprocal_approx_fast(
                                        dnr[:], dn[:])
                                    db = p3d.tile([64, 512], F32, tag="db",
                                                  name=f"db{h}_{tcx}")
                                    nc.gpsimd.partition_broadcast(
                                        db[:], dnr[:])
                                    nc.vector.tensor_mul(
                                        yT[po:po + 64, g, qsl],
                                        py[0:64, :], db[:])
                        cur = nxt
                    # stream finished token range of this head-pair out
                    psl = slice(pair[0] * 512, (pair[1] + 1) * 512)
                    nc.sync.dma_start(out=yT_d[:, g, psl], in_=yT[:, g, psl])
    nc.compile()
    return nc


# --------------------------------------------------------------------------
# Launch 2: c_proj + residual + LN2 + MLP + residual
# --------------------------------------------------------------------------
def build_l2(s_act: float):
    nc = bacc.Bacc("TRN2", target_bir_lowering=False, debug=False,
                   num_devices=N_CORES)
    yin_d = nc.dram_tensor("yin", [128, 8, 512], BF16, kind="ExternalInput")
    pw_d = nc.dram_tensor("pwT", [128, 8, 1024], BF16, kind="ExternalInput")
    xs_d = nc.dram_tensor("xs", [TS, C], F32, kind="ExternalInput")
    pb_d = nc.dram_tensor("pb", [1, C], F32, kind="ExternalInput")
    fb2_d = nc.dram_tensor("fb2", [1, C], F32, kind="ExternalInput")
    ab_d = nc.dram_tensor("abias", [128, 32], F32, kind="ExternalInput")
    fcw_d = nc.dram_tensor("fcwT", [8, 128, 8, 512], BF16, kind="ExternalInput")
    fc2w_d = nc.dram_tensor("fc2wT", [8, 128, 4, C], BF16, kind="ExternalInput")
    idb_d = nc.dram_tensor("identb", [128, 128], BF16, kind="ExternalInput")
    out_d = nc.dram_tensor("out", [TS, C], F32, kind="ExternalOutput")

    n_ttiles = TS // 128    # 4

    with tile.TileContext(nc) as tc, ExitStack() as ctx:
        consts = ctx.enter_context(tc.tile_pool(name="consts", bufs=1))
        identb = consts.tile([128, 128], BF16)
        nc.sync.dma_start(out=identb[:], in_=idb_d[:])
        eps_sb = consts.tile([128, 1], F32)
        nc.vector.memset(eps_sb[:], 1e-5)
        pb_row = consts.tile([1, C], F32)
        nc.sync.dma_start(out=pb_row[:], in_=pb_d[:])
        pb_b = consts.tile([128, C], F32)
        nc.gpsimd.partition_broadcast(pb_b[:], pb_row[:])
        fb2_row = consts.tile([1, C], F32)
        nc.sync.dma_start(out=fb2_row[:], in_=fb2_d[:])
        fb2_b = consts.tile([128, C], F32)
        nc.gpsimd.partition_broadcast(fb2_b[:], fb2_row[:])
        ab_sb = consts.tile([128, 32], F32)
        nc.sync.dma_start(out=ab_sb[:], in_=ab_d[:])

        big = ctx.enter_context(tc.tile_pool(name="big", bufs=1))
        h2T = big.tile([128, 8, TS], BF16)         # 8KB/p
        x2pb = big.tile([128, n_ttiles, C], F32)   # x2 + fc2 bias, 16KB/p
        actT = big.tile([128, 32, TS], BF16)       # 32KB/p
        yin_sb = big.tile([128, 8, 512], BF16)
        nc.sync.dma_start(out=yin_sb[:], in_=yin_d[:])
        pw_sb = big.tile([128, 8, 1024], BF16)
        nc.sync.dma_start(out=pw_sb[:], in_=pw_d[:])

        # ---- P1: c_proj + residual + proj bias, LN2, transpose ----
        with tc.tile_pool(name="q1", bufs=3) as q1, \
             tc.tile_pool(name="q1s", bufs=4) as q1s, \
             tc.tile_pool(name="q1psum", bufs=2, space="PSUM") as q1p, \
             tc.tile_pool(name="q1pt", bufs=2, space="PSUM") as q1pt:
            for tt in range(n_ttiles):
                x2 = q1.tile([128, C], F32, tag="x2")
                px2 = [q1p.tile([128, 512], F32, tag=f"px{ch}", name=f"px{tt}_{ch}")
                       for ch in range(2)]
                for k in range(8):
                    for ch in range(2):
                        nc.tensor.matmul(
                            px2[ch][:], yin_sb[:, k, tt * 128:(tt + 1) * 128],
                            pw_sb[:, k, ch * 512:(ch + 1) * 512],
                            start=(k == 0), stop=(k == 7),
                            skip_group_check=True)
                xst = q1.tile([128, C], F32, tag="xs")
                nc.sync.dma_start(out=xst[:], in_=xs_d[tt * 128:(tt + 1) * 128, :])
                xpb = q1.tile([128, C], F32, tag="xpb")
                nc.gpsimd.tensor_add(xpb[:], xst[:], pb_b[:])
                for ch in range(2):
                    csl = slice(ch * 512, (ch + 1) * 512)
                    nc.vector.tensor_add(x2[:, csl], px2[ch][:], xpb[:, csl])
                nc.vector.tensor_add(x2pb[:, tt, :], x2[:], fb2_b[:])
                stats = q1s.tile([128, 2, 6], F32)
                x2g = x2[:].rearrange("p (g d) -> p g d", g=2)
                nc.vector.bn_stats(out=stats[:, 0, :], in_=x2g[:, 0, :])
                nc.vector.bn_stats(out=stats[:, 1, :], in_=x2g[:, 1, :])
                mv = q1s.tile([128, 2], F32)
                nc.vector.bn_aggr(out=mv[:], in_=stats[:])
                sd = q1s.tile([128, 1], F32, tag="sd")
                nc.scalar.activation(out=sd[:], in_=mv[:, 1:2], func=AF.Sqrt,
                                     bias=eps_sb[:], scale=1.0)
                rstd = q1s.tile([128, 1], F32)
                nc.vector.reciprocal(rstd[:], sd[:])
                h2 = q1.tile([128, C], BF16, tag="h2")
                nc.vector.tensor_scalar(out=h2[:], in0=x2[:],
                                        scalar1=mv[:, 0:1], scalar2=rstd[:],
                                        op0=ALU.subtract, op1=ALU.mult)
                for c in range(8):
                    pt = q1pt.tile([128, 128], BF16)
                    nc.tensor.transpose(pt[:], h2[:, c * 128:(c + 1) * 128],
                                        identb[:])
                    nc.vector.tensor_copy(h2T[:, c, tt * 128:(tt + 1) * 128],
                                          pt[:])

        # ---- P2: c_fc + gaussian activation via Derivative_Erf ----
        with tc.tile_pool(name="q2w", bufs=2) as q2w, \
             tc.tile_pool(name="q2psum", bufs=3, space="PSUM") as q2p:
            for hc in range(8):
                wt = q2w.tile([128, 8, 512], BF16)
                nc.sync.dma_start(out=wt[:], in_=fcw_d[hc])
                for ht in range(4):
                    pu = q2p.tile([128, TS], F32)
                    for c in range(8):
                        nc.tensor.matmul(
                            pu[:], wt[:, c, ht * 128:(ht + 1) * 128],
                            h2T[:, c, :], start=(c == 0), stop=(c == 7))
                    hi = hc * 4 + ht
                    nc.scalar.activation(out=actT[:, hi, :], in_=pu[:],
                                         func=AF.Derivative_Erf,
                                         bias=ab_sb[:, hi:hi + 1], scale=s_act)

        # ---- P3: c_fc2 + residual ----
        with tc.tile_pool(name="q3w", bufs=2) as q3w, \
             tc.tile_pool(name="q3o", bufs=3) as q3o, \
             tc.tile_pool(name="q3psum", bufs=1, space="PSUM") as q3p:
            po_tiles = []
            for tt in range(n_ttiles):
                row = []
                for co in range(2):
                    po_t = q3p.tile([128, 512], F32, tag=f"o{tt}{co}",
                                    name=f"po{tt}{co}")
                    row.append(po_t)
                po_tiles.append(row)
            for kr in range(8):
                w2 = q3w.tile([128, 4, C], BF16)
                nc.sync.dma_start(out=w2[:], in_=fc2w_d[kr])
                for tt in range(n_ttiles):
                    for k4 in range(4):
                        k = kr * 4 + k4
                        for co in range(2):
                            nc.tensor.matmul(
                                po_tiles[tt][co][:],
                                actT[:, k, tt * 128:(tt + 1) * 128],
                                w2[:, k4, co * 512:(co + 1) * 512],
                                start=(kr == 0 and k4 == 0),
                                stop=(kr == 7 and k4 == 3),
                                skip_group_check=True)
            for tt in range(n_ttiles):
                for co in range(2):
                    ot = q3o.tile([128, 512], F32)
                    nc.vector.tensor_add(ot[:], po_tiles[tt][co][:],
                                         x2pb[:, tt, co * 512:(co + 1) * 512])
                    nc.sync.dma_start(
                        out=out_d[tt * 128:(tt + 1) * 128,
                                  co * 512:(co + 1) * 512], in_=ot[:])
    nc.compile()
    return nc


# --------------------------------------------------------------------------
# Host-side orchestration
# --------------------------------------------------------------------------
_PROG_CACHE = {}


def _get_prog(key, builder, *args):
    if key not in _PROG_CACHE:
        _PROG_CACHE[key] = builder(*args)
    return _PROG_CACHE[key]


def _causal_masks4():
    s = np.arange(128)[:, None]
    t = np.arange(512)[None, :]
    ms = [((s + 128 * m) <= t).astype(np.float32) for m in range(4)]
    return np.ascontiguousarray(np.stack(ms, axis=1))  # [128, 4, 512]


def _perm(w, tiles, width):
    """[tiles*128, width] -> [128, tiles, width] (partition-major for DMA)."""
    return np.ascontiguousarray(w.reshape(tiles, 128, width).transpose(1, 0, 2))


def _bf(a):
    return np.ascontiguousarray(np.asarray(a, dtype=np.float32).astype(BF))


def kernel(x, ln1_w, ln1_b, attn_w, attn_b, proj_w, proj_b,
           ln2_w, ln2_b, fc_w, fc_b, fc2_w, fc2_b,
           mu, sigma, gamma, beta, n_head):
    x = np.asarray(x, dtype=np.float32)
    attn_w = np.asarray(attn_w, dtype=np.float32)
    attn_b = np.asarray(attn_b, dtype=np.float32)
    proj_w = np.asarray(proj_w, dtype=np.float32)
    proj_b = np.asarray(proj_b, dtype=np.float32)
    fc_w = np.asarray(fc_w, dtype=np.float32)
    fc_b = np.asarray(fc_b, dtype=np.float32)
    fc2_w = np.asarray(fc2_w, dtype=np.float32)
    fc2_b = np.asarray(fc2_b, dtype=np.float32)
    ln1_w = np.asarray(ln1_w, dtype=np.float32)
    ln1_b = np.asarray(ln1_b, dtype=np.float32)
    ln2_w = np.asarray(ln2_w, dtype=np.float32)
    ln2_b = np.asarray(ln2_b, dtype=np.float32)
    mu = float(mu)
    sigma = float(sigma)
    gamma = float(gamma)
    beta = float(beta)
    n_head = int(n_head)

    B = x.shape[0]
    assert x.shape == (B, T, C) and B == 2 and n_head == 16

    _install_compile_cache()
    trace = bool(int(os.environ.get("BASS_KERNEL_TRACE", "0")))

    sig = abs(sigma) + 1e-8
    s_act = float(1.0 / (np.sqrt(2.0) * sig))

    # Fold LN affine params into the consuming projection weights (host-side).
    attn_w_eff = attn_w * ln1_w[None, :]
    attn_b_eff = attn_b + attn_w @ ln1_b
    fc_w_eff = fc_w * ln2_w[None, :]
    fc_b_eff = fc_b + fc_w @ ln2_b

    # ---- launch 1 ----
    nc1 = _get_prog(("l1",), build_l1)
    masks = _bf(_causal_masks4())
    vones = _bf(np.ones((128, 1), dtype=np.float32))
    ident = np.eye(128, dtype=np.float32)
    in_maps1 = []
    for c in range(N_CORES):
        b, hg = c // 4, c % 4
        q_rows = attn_w_eff[hg * 256:(hg + 1) * 256] * 0.125
        k_rows = attn_w_eff[C + hg * 256:C + (hg + 1) * 256]
        v_rows = attn_w_eff[2 * C + hg * 256:2 * C + (hg + 1) * 256]
        wqk = np.concatenate([q_rows, k_rows], axis=0)   # [512, 1024]
        bqk = np.concatenate([attn_b_eff[hg * 256:(hg + 1) * 256] * 0.125,
                              attn_b_eff[C + hg * 256:C + (hg + 1) * 256]])
        bv = attn_b_eff[2 * C + hg * 256:2 * C + (hg + 1) * 256]
        m = {
            "xT": _bf(_perm(np.ascontiguousarray(x[b].T), 8, T)
                      .reshape(128, 8, 4, 512).transpose(2, 0, 1, 3)),
            "wqkT": _bf(_perm(np.ascontiguousarray(wqk.T), 8, 512)),
            "wvT": _bf(_perm(np.ascontiguousarray(v_rows.T), 8, 256)),
            "nws": np.ascontiguousarray(-wqk.sum(axis=1).reshape(1, 4, 128)),
            "bqk": np.ascontiguousarray(bqk.reshape(1, 4, 128)),
            "bv": np.ascontiguousarray(bv[None, :]),
            "nwsv": np.ascontiguousarray(-v_rows.sum(axis=1)[None, :]),
            "masks": masks,
            "vones": vones,
            "ident": ident,
        }
        in_maps1.append(m)
    res1 = run_bass_kernel_spmd(nc1, in_maps1, list(range(N_CORES)), trace=trace)
    if res1.exec_time_ns is not None:
        LAST_EXEC_NS["l1"] = res1.exec_time_ns
    # yT [128, 2, 2048]: head h local = 2*cl + (po//64); reorder to [256, 2048]
    ystrips = []
    for c in range(N_CORES):
        yt = np.asarray(res1.results[c]["yT"])
        ystrips.append(np.ascontiguousarray(
            yt.reshape(2, 64, 2, T).transpose(2, 0, 1, 3).reshape(256, T)))

    # ---- launch 2 ----
    nc2 = _get_prog(("l2", s_act), build_l2, s_act)
    fc2w_eff = (gamma * np.sqrt(np.pi) / 2.0 * fc2_w).T      # [4096, 1024]
    fb2_eff = fc2_b + beta * fc2_w.sum(axis=1)
    abias = ((fc_b_eff - mu) * s_act).reshape(32, 128).T     # [128, 32]
    fcwT_p = _perm(np.ascontiguousarray(fc_w_eff.T), 8, HID)      # [128,8,4096]
    fcw_chunks = _bf(
        fcwT_p.reshape(128, 8, 8, 512).transpose(2, 0, 1, 3))     # [8,128,8,512]
    fc2wT_p = _perm(np.ascontiguousarray(fc2w_eff), 32, C)        # [128,32,1024]
    fc2w_chunks = _bf(
        fc2wT_p.reshape(128, 8, 4, C).transpose(1, 0, 2, 3))      # [8,128,4,1024]
    pwT = _bf(_perm(np.ascontiguousarray(proj_w.T), 8, C))        # [128,8,1024]
    identb = _bf(np.eye(128, dtype=np.float32))
    in_maps2 = []
    for c in range(N_CORES):
        b, slc = c // 4, c % 4
        t0 = slc * TS
        yin = np.concatenate(
            [ystrips[b * 4 + g][:, t0:t0 + TS] for g in range(4)], axis=0)
        m = {
            "yin": np.ascontiguousarray(
                yin.reshape(8, 128, TS).transpose(1, 0, 2)),
            "pwT": pwT,
            "xs": np.ascontiguousarray(x[b, t0:t0 + TS]),
            "pb": proj_b[None, :],
            "fb2": np.ascontiguousarray(fb2_eff[None, :]),
            "abias": np.ascontiguousarray(abias),
            "fcwT": fcw_chunks,
            "fc2wT": fc2w_chunks,
            "identb": identb,
        }
        in_maps2.append(m)
    res2 = run_bass_kernel_spmd(nc2, in_maps2, list(range(N_CORES)), trace=trace)
    if res2.exec_time_ns is not None:
        LAST_EXEC_NS["l2"] = res2.exec_time_ns

    out = np.empty((B, T, C), dtype=np.float32)
    for c in range(N_CORES):
        b, slc = c // 4, c % 4
        out[b, slc * TS:(slc + 1) * TS] = res2.results[c]["out"]
    return out


# revision 20
# speedup vs baseline: 1.4213x; 1.0248x over previous
"""Trainium2 Bass kernel for a dense transformer block (B=2, T=2048, C=1024, nh=16, H=4096).

Two SPMD launches over 8 NeuronCores (no device collectives):

  Launch 1 (head-parallel attention): cores 0-3 <- batch 0, cores 4-7 <- batch 1;
    each core does 4 heads over the full sequence. x arrives pre-transposed
    (feature-major) in bf16. LN1 statistics via ones-matmuls; the -mu*wsum and
    +bias LN terms are folded into the QKV PSUM accumulation as two rank-1
    matmuls, so the epilogue is a single VectorE multiply by the broadcast
    rstd. Q/K/V and attention run in bf16 (fp32 PSUM accumulate). The paired
    heads (even/odd) live in partition halves of the same feature tiles, so
    QK^T runs as 4 concurrent quadrant matmuls (tile_position packing, 2x).
    V carries a ones column so the softmax denominator falls out of the AV
    matmul; the divide happens right after each AV accumulation finishes.
    Output: un-projected attention y (feature-major, bf16) [128, 2, 2048].

  Host: pure re-slicing (no arithmetic on activations).

  Launch 2 (token-parallel): each core takes a 512-token slice: c_proj over
    the concatenated head outputs (contracts over all 1024 y-features, so no
    separate partial reduction is needed) + residual + proj_b -> LN2
    (bn_stats) -> transpose -> c_fc (bf16 weights) -> gaussian activation as a
    single Derivative_Erf pass (d/dx erf = 2/sqrt(pi) e^{-x^2}; sqrt(pi)/2,
    gamma, beta, mu, sigma and biases are folded host-side) -> c_fc2 (bf16)
    -> + residual -> output slice.
"""

import hashlib
import os
import shutil
from contextlib import ExitStack

import numpy as np
import ml_dtypes

import concourse.bass as bass
import concourse.tile as tile
from concourse import bacc, mybir
from concourse.bass_utils import run_bass_kernel_spmd

F32 = mybir.dt.float32
F32R = mybir.dt.float32r
BF16 = mybir.dt.bfloat16
AF = mybir.ActivationFunctionType
ALU = mybir.AluOpType
BF = ml_dtypes.bfloat16

N_CORES = 8
T = 2048          # tokens per batch
C = 1024          # model dim
NH_LOC = 4        # heads per core (launch 1)
HS = 64           # head size
HID = 4096        # mlp hidden
TS = 512          # tokens per core (launch 2)

LAST_EXEC_NS = {}  # launch name -> exec_time_ns (filled when tracing enabled)

_CACHE_DIR = "/tmp/neff_cache"


def _install_compile_cache():
    import concourse.bass2jax as b2j

    if getattr(b2j, "_neff_cache_installed", False):
        return
    real = b2j.compile_bir_kernel

    def cached(bir_json, tmpdir, neff_name="file.neff"):
        os.makedirs(_CACHE_DIR, exist_ok=True)
        h = hashlib.sha256(bir_json).hexdigest()
        cpath = os.path.join(_CACHE_DIR, h + ".neff")
        out = os.path.join(tmpdir, neff_name)
        if os.path.exists(cpath):
            shutil.copyfile(cpath, out)
            return out
        res = real(bir_json, tmpdir, neff_name)
        shutil.copyfile(res, cpath)
        return res

    b2j.compile_bir_kernel = cached
    b2j._neff_cache_installed = True


# --------------------------------------------------------------------------
# Launch 1: LN1 + QKV + causal attention (4 heads), un-projected y out
# --------------------------------------------------------------------------
def build_l1():
    nc = bacc.Bacc("TRN2", target_bir_lowering=False, debug=False,
                   num_devices=N_CORES)
    xT_d = nc.dram_tensor("xT", [4, 128, 8, 512], BF16, kind="ExternalInput")
    wqk_d = nc.dram_tensor("wqkT", [128, 8, 512], BF16, kind="ExternalInput")
    wv_d = nc.dram_tensor("wvT", [128, 8, 256], BF16, kind="ExternalInput")
    nws_d = nc.dram_tensor("nws", [1, 4, 128], F32R, kind="ExternalInput")
    bqk_d = nc.dram_tensor("bqk", [1, 4, 128], F32R, kind="ExternalInput")
    bv_d = nc.dram_tensor("bv", [1, 256], F32, kind="ExternalInput")
    nwsv_d = nc.dram_tensor("nwsv", [1, 256], F32, kind="ExternalInput")
    mask_d = nc.dram_tensor("masks", [128, 4, 1024], BF16, kind="ExternalInput")
    vones_d = nc.dram_tensor("vones", [128, 1], BF16, kind="ExternalInput")
    id_d = nc.dram_tensor("ident", [128, 128], F32, kind="ExternalInput")
    yT_d = nc.dram_tensor("yT", [128, 2, T], BF16, kind="ExternalOutput")

    n_tc = T // 512              # 4 query chunks

    with tile.TileContext(nc) as tc, ExitStack() as ctx:
        consts = ctx.enter_context(tc.tile_pool(name="consts", bufs=1))
        vones_sb = consts.tile([128, 1], BF16)
        nc.sync.dma_start(out=vones_sb[:], in_=vones_d[:])
        eps_sb = consts.tile([128, 1], F32)
        nc.vector.memset(eps_sb[:], 1e-5)
        nws_sb = consts.tile([1, 4, 128], F32R)
        nc.sync.dma_start(out=nws_sb[:], in_=nws_d[:])
        bqk_sb = consts.tile([1, 4, 128], F32R)
        nc.sync.dma_start(out=bqk_sb[:], in_=bqk_d[:])
        ident = consts.tile([128, 128], F32)
        nc.sync.dma_start(out=ident[:], in_=id_d[:])
        bv_row = consts.tile([1, 256], F32)
        nc.sync.dma_start(out=bv_row[:], in_=bv_d[:])
        bv_b = consts.tile([128, 256], F32)
        nc.gpsimd.partition_broadcast(bv_b[:], bv_row[:])
        nwsv_row = consts.tile([1, 256], F32)
        nc.sync.dma_start(out=nwsv_row[:], in_=nwsv_d[:])
        nwsv_b = consts.tile([128, 256], F32)
        nc.gpsimd.partition_broadcast(nwsv_b[:], nwsv_row[:])

        big2 = ctx.enter_context(tc.tile_pool(name="big2", bufs=1))
        qkT = big2.tile([128, 4, T], BF16)     # Q feats (tiles 0,1), K feats (2,3)
        v_sb = big2.tile([128, 16, NH_LOC, 65], BF16)  # col 64 = ones
        yT = big2.tile([128, 2, T], BF16)
        nc.vector.memset(v_sb[:, :, :, 64:65], 1.0)

        wpool = ctx.enter_context(tc.tile_pool(name="wpool", bufs=1))
        wqk_sb = wpool.tile([128, 8, 512], BF16)
        nc.sync.dma_start(out=wqk_sb[:], in_=wqk_d[:])
        wv_sb = wpool.tile([128, 8, 256], BF16)
        nc.sync.dma_start(out=wv_sb[:], in_=wv_d[:])

        # ---- P1 per 512-token chunk: raw QKV matmuls on un-normalized xT;
        # LN folded in as rank-1 PSUM corrections + one epilogue multiply:
        #   qkv[f,t] = r[t]*(raw[f,t] - mu[t]*wsum[f] + sd[t]*b[f])
        with tc.tile_pool(name="p1", bufs=3) as p1, \
             tc.tile_pool(name="p1sq", bufs=3) as p1sq, \
             tc.tile_pool(name="p1sl", bufs=2) as p1sl, \
             tc.tile_pool(name="p1b", bufs=2) as p1b, \
             tc.tile_pool(name="p1c", bufs=2) as p1c, \
             tc.tile_pool(name="p1r", bufs=8) as p1r, \
             tc.tile_pool(name="p1psum", bufs=1, space="PSUM") as p1p, \
             tc.tile_pool(name="ptp", bufs=2, space="PSUM") as ptp, \
             tc.tile_pool(name="p2psum", bufs=2, space="PSUM") as p2p:
            for tch in range(n_tc):
                sl = slice(tch * 512, (tch + 1) * 512)
                slab = p1sl.tile([128, 8, 512], BF16, tag="slab")
                nc.sync.dma_start(out=slab[:, 0:4, :], in_=xT_d[tch, :, 0:4, :])
                nc.sync.dma_start(out=slab[:, 4:8, :], in_=xT_d[tch, :, 4:8, :])
                ps_sum = p1p.tile([1, 512], F32, tag="s")
                ps_sq = p1p.tile([1, 512], F32, tag="q")
                for c in range(8):
                    sq = p1sq.tile([128, 512], BF16, tag="sq")
                    nc.scalar.activation(out=sq[:], in_=slab[:, c, :],
                                         func=AF.Square)
                    nc.tensor.matmul(ps_sum[:], vones_sb[:], slab[:, c, :],
                                     start=(c == 0), stop=(c == 7),
                                     skip_group_check=True)
                    nc.tensor.matmul(ps_sq[:], vones_sb[:], sq[:],
                                     start=(c == 0), stop=(c == 7),
                                     skip_group_check=True)
                # row math: mu, sd, rstd (rows live on partition 0).
                # mrow/sdr are F32R because they feed matmuls as rhs.
                srow = p1r.tile([1, 512], F32, tag="row")
                nc.vector.tensor_copy(srow[:], ps_sum[:])
                qrow = p1r.tile([1, 512], F32, tag="row")
                nc.vector.tensor_copy(qrow[:], ps_sq[:])
                mrow = p1r.tile([1, 512], F32R, tag="rowr")
                nc.vector.tensor_scalar(out=mrow[:], in0=srow[:], scalar1=1.0 / C,
                                        scalar2=None, op0=ALU.mult)
                msq = p1r.tile([1, 512], F32, tag="row")
                nc.vector.tensor_mul(msq[:], mrow[:], mrow[:])
                vrow = p1r.tile([1, 512], F32, tag="row")
                nc.vector.scalar_tensor_tensor(out=vrow[:], in0=qrow[:],
                                               scalar=1.0 / C, in1=msq[:],
                                               op0=ALU.mult, op1=ALU.subtract)
                sdr = p1r.tile([1, 512], F32R, tag="rowr")
                nc.scalar.activation(out=sdr[:], in_=vrow[:], func=AF.Sqrt,
                                     bias=eps_sb[0:1], scale=1.0)
                rrow = p1r.tile([1, 512], F32, tag="row")
                nc.vector.reciprocal_approx_fast(rrow[:],
                                                 sdr[:].bitcast(F32))
                rmurow = p1r.tile([1, 512], F32, tag="row")
                nc.vector.tensor_mul(rmurow[:], rrow[:], mrow[:])
                rb = p1b.tile([128, 512], F32, tag="rb")
                nc.gpsimd.partition_broadcast(rb[:], rrow[:])
                # token-major per-partition columns of r and r*mu (for V)
                rcol = p1c.tile([128, 4], F32, tag="rcol")
                rmucol = p1c.tile([128, 4], F32, tag="rmucol")
                for j in range(4):
                    tp1 = ptp.tile([128, 1], F32, tag="tp")
                    nc.tensor.transpose(tp1[:], rrow[0:1, j * 128:(j + 1) * 128],
                                        ident[0:1, 0:1])
                    nc.vector.tensor_copy(rcol[:, j:j + 1], tp1[:])
                    tp2 = ptp.tile([128, 1], F32, tag="tp")
                    nc.tensor.transpose(tp2[:], rmurow[0:1, j * 128:(j + 1) * 128],
                                        ident[0:1, 0:1])
                    nc.vector.tensor_copy(rmucol[:, j:j + 1], tp2[:])
                # raw Q/K projections + rank-1 LN corrections + epilogue
                for f in range(4):
                    ps = p2p.tile([128, 512], F32, tag="qk")
                    for c in range(8):
                        nc.tensor.matmul(
                            ps[:], wqk_sb[:, c, f * 128:(f + 1) * 128],
                            slab[:, c, :], start=(c == 0), stop=False,
                            skip_group_check=True)
                    nc.tensor.matmul(ps[:], nws_sb[:, f, :], mrow[:],
                                     start=False, stop=False,
                                     skip_group_check=True)
                    nc.tensor.matmul(ps[:], bqk_sb[:, f, :], sdr[:],
                                     start=False, stop=True,
                                     skip_group_check=True)
                    nc.vector.tensor_mul(qkT[:, f, sl], ps[:], rb[:])
                # raw V + LN epilogue (token-major)
                for tt4 in range(4):
                    tt = tch * 4 + tt4
                    ps = p2p.tile([128, 256], F32, tag="v")
                    for c in range(8):
                        nc.tensor.matmul(
                            ps[:], slab[:, c, tt4 * 128:(tt4 + 1) * 128],
                            wv_sb[:, c, :], start=(c == 0), stop=(c == 7))
                    corr = p1.tile([128, 256], F32, tag="corr")
                    nc.vector.scalar_tensor_tensor(
                        out=corr[:], in0=nwsv_b[:],
                        scalar=rmucol[:, tt4:tt4 + 1], in1=bv_b[:],
                        op0=ALU.mult, op1=ALU.add)
                    nc.vector.scalar_tensor_tensor(
                        out=v_sb[:, tt, :, 0:64],
                        in0=ps[:].rearrange("p (h d) -> p h d", h=NH_LOC),
                        scalar=rcol[:, tt4:tt4 + 1],
                        in1=corr[:].rearrange("p (h d) -> p h d", h=NH_LOC),
                        op0=ALU.mult, op1=ALU.add)

        # ---- P3: attention; paired heads (even/odd partition halves) share
        # quadrant-packed QK matmuls and one wide exp per (s, tcx).
        with tc.tile_pool(name="p3consts", bufs=1) as p3c, \
             tc.tile_pool(name="p3a", bufs=4) as p3a, \
             tc.tile_pool(name="p3y", bufs=4) as p3y, \
             tc.tile_pool(name="p3d", bufs=4) as p3d, \
             tc.tile_pool(name="p3sc", bufs=2, space="PSUM") as p3sc, \
             tc.tile_pool(name="p3py", bufs=1, space="PSUM") as p3py:
            mask_sb = p3c.tile([128, 4, 1024], BF16)
            nc.sync.dma_start(out=mask_sb[:], in_=mask_d[:])
            for g in range(2):
                hA, hB = 2 * g, 2 * g + 1
                qf = g
                kf = 2 + g
                for pair in ((0, 1), (2, 3)):
                    smax = 4 * pair[1] + 4
                    pys = {}
                    for tcx in pair:
                        for h, po in ((hA, 0), (hB, 64)):
                            pys[(h, tcx)] = p3py.tile(
                                [65, 512], F32, tag=f"py{h % 2}{tcx % 2}",
                                name=f"py{h}_{tcx}")

                    def emit_qk(s):
                        tiles = []
                        for tcx in pair:
                            if s > 4 * tcx + 3:
                                continue
                            qsl = slice(tcx * 512, (tcx + 1) * 512)
                            sc = p3sc.tile([128, 1024], F32, tag="sc",
                                           name=f"sc{g}_{s}_{tcx}")
                            k0 = s * 128
                            # issue order pairs disjoint quadrants so the
                            # sub-arrays overlap: (A-lo | B-hi), (A-hi | B-lo)
                            for hj, kj in ((0, 0), (1, 1), (0, 1), (1, 0)):
                                po = hj * 64
                                nc.tensor.matmul(
                                    sc[kj * 64:(kj + 1) * 64,
                                       hj * 512:(hj + 1) * 512],
                                    qkT[po:po + 64, kf,
                                        k0 + kj * 64:k0 + (kj + 1) * 64],
                                    qkT[po:po + 64, qf, qsl],
                                    start=True, stop=True,
                                    skip_group_check=True,
                                    tile_position=(po, kj * 64))
                            at = p3a.tile([128, 1024], BF16, tag="at",
                                          name=f"at{g}_{s}_{tcx}")
                            nc.scalar.activation(out=at[:], in_=sc[:],
                                                 func=AF.Exp)
                            if tcx == s // 4:
                                atm = p3a.tile([128, 1024], BF16, tag="atm",
                                               name=f"atm{g}_{s}_{tcx}")
                                nc.vector.tensor_mul(atm[:], at[:],
                                                     mask_sb[:, s % 4, :])
                                at = atm
                            tiles.append((tcx, at))
                        return tiles

                    cur = emit_qk(0)
                    for s in range(smax):
                        nxt = emit_qk(s + 1) if s + 1 < smax else []
                        for tcx, at in cur:
                            qsl = slice(tcx * 512, (tcx + 1) * 512)
                            for h, po in ((hA, 0), (hB, 64)):
                                py = pys[(h, tcx)]
                                nc.tensor.matmul(
                                    py[:], v_sb[:, s, h, :],
                                    at[:, (po // 64) * 512:(po // 64) * 512 + 512],
                                    start=(s == 0), stop=(s == 4 * tcx + 3),
                                    skip_group_check=True)
                                if s == 4 * tcx + 3:
                                    dcp = p3y.tile([65, 512], F32, tag="dcp",
                                                   name=f"dcp{h}_{tcx}")
                                    nc.vector.tensor_copy(dcp[64:65, :],
                                                          py[64:65, :])
                                    dn = p3d.tile([1, 512], F32, tag="dn",
                                                  name=f"dn{h}_{tcx}")
                                    nc.sync.dma_start(out=dn[:],
                                                      in_=dcp[64:65, :])
                                    dnr = p3d.tile([1, 512], F32, tag="dnr",
                                                   name=f"dnr{h}_{tcx}")
                                    nc.vector.reciprocal_approx_fast(
                                        dnr[:], dn[:])
                                    db = p3d.tile([64, 512], F32, tag="db",
                                                  name=f"db{h}_{tcx}")
                                    nc.gpsimd.partition_broadcast(
                                        db[:], dnr[:])
                                    nc.vector.tensor_mul(
                                        yT[po:po + 64, g, qsl],
                                        py[0:64, :], db[:])
                        cur = nxt
                    # stream finished token range of this head-pair out
                    psl = slice(pair[0] * 512, (pair[1] + 1) * 512)
                    nc.sync.dma_start(out=yT_d[:, g, psl], in_=yT[:, g, psl])
    nc.compile()
    return nc


# --------------------------------------------------------------------------
# Launch 2: c_proj + residual + LN2 + MLP + residual
# --------------------------------------------------------------------------
def build_l2(s_act: float):
    nc = bacc.Bacc("TRN2", target_bir_lowering=False, debug=False,
                   num_devices=N_CORES)
    yin_d = nc.dram_tensor("yin", [128, 8, 512], BF16, kind="ExternalInput")
    pw_d = nc.dram_tensor("pwT", [128, 8, 1024], BF16, kind="ExternalInput")
    xs_d = nc.dram_tensor("xs", [TS, C], F32, kind="ExternalInput")
    pb_d = nc.dram_tensor("pb", [1, C], F32, kind="ExternalInput")
    fb2_d = nc.dram_tensor("fb2", [1, C], F32, kind="ExternalInput")
    ab_d = nc.dram_tensor("abias", [128, 32], F32, kind="ExternalInput")
    fcw_d = nc.dram_tensor("fcwT", [8, 128, 8, 512], BF16, kind="ExternalInput")
    fc2w_d = nc.dram_tensor("fc2wT", [8, 2, 128, 4, 512], BF16,
                            kind="ExternalInput")
    idb_d = nc.dram_tensor("identb", [128, 128], BF16, kind="ExternalInput")
    out_d = nc.dram_tensor("out", [TS, C], F32, kind="ExternalOutput")

    n_ttiles = TS // 128    # 4

    with tile.TileContext(nc) as tc, ExitStack() as ctx:
        consts = ctx.enter_context(tc.tile_pool(name="consts", bufs=1))
        identb = consts.tile([128, 128], BF16)
        nc.sync.dma_start(out=identb[:], in_=idb_d[:])
        eps_sb = consts.tile([128, 1], F32)
        nc.vector.memset(eps_sb[:], 1e-5)
        pb_row = consts.tile([1, C], F32)
        nc.sync.dma_start(out=pb_row[:], in_=pb_d[:])
        pb_b = consts.tile([128, C], F32)
        nc.gpsimd.partition_broadcast(pb_b[:], pb_row[:])
        fb2_row = consts.tile([1, C], F32)
        nc.sync.dma_start(out=fb2_row[:], in_=fb2_d[:])
        fb2_b = consts.tile([128, C], F32)
        nc.gpsimd.partition_broadcast(fb2_b[:], fb2_row[:])
        ab_sb = consts.tile([128, 32], F32)
        nc.sync.dma_start(out=ab_sb[:], in_=ab_d[:])

        big = ctx.enter_context(tc.tile_pool(name="big", bufs=1))
        h2T = big.tile([128, 8, TS], BF16)         # 8KB/p
        x2pb = big.tile([128, n_ttiles, C], F32)   # x2 + fc2 bias, 16KB/p
        actT = big.tile([128, 32, TS], BF16)       # 32KB/p
        yin_sb = big.tile([128, 8, 512], BF16)
        nc.sync.dma_start(out=yin_sb[:], in_=yin_d[:])
        pw_sb = big.tile([128, 8, 1024], BF16)
        nc.sync.dma_start(out=pw_sb[:], in_=pw_d[:])

        # ---- P1: c_proj + residual + proj bias, LN2, transpose ----
        with tc.tile_pool(name="q1", bufs=3) as q1, \
             tc.tile_pool(name="q1s", bufs=4) as q1s, \
             tc.tile_pool(name="q1psum", bufs=2, space="PSUM") as q1p, \
             tc.tile_pool(name="q1pt", bufs=2, space="PSUM") as q1pt:
            for tt in range(n_ttiles):
                x2 = q1.tile([128, C], F32, tag="x2")
                px2 = [q1p.tile([128, 512], F32, tag=f"px{ch}", name=f"px{tt}_{ch}")
                       for ch in range(2)]
                for k in range(8):
                    for ch in range(2):
                        nc.tensor.matmul(
                            px2[ch][:], yin_sb[:, k, tt * 128:(tt + 1) * 128],
                            pw_sb[:, k, ch * 512:(ch + 1) * 512],
                            start=(k == 0), stop=(k == 7),
                            skip_group_check=True)
                xst = q1.tile([128, C], F32, tag="xs")
                nc.sync.dma_start(out=xst[:], in_=xs_d[tt * 128:(tt + 1) * 128, :])
                xpb = q1.tile([128, C], F32, tag="xpb")
                nc.gpsimd.tensor_add(xpb[:], xst[:], pb_b[:])
                for ch in range(2):
                    csl = slice(ch * 512, (ch + 1) * 512)
                    nc.vector.tensor_add(x2[:, csl], px2[ch][:], xpb[:, csl])
                nc.vector.tensor_add(x2pb[:, tt, :], x2[:], fb2_b[:])
                stats = q1s.tile([128, 2, 6], F32)
                x2g = x2[:].rearrange("p (g d) -> p g d", g=2)
                nc.vector.bn_stats(out=stats[:, 0, :], in_=x2g[:, 0, :])
                nc.vector.bn_stats(out=stats[:, 1, :], in_=x2g[:, 1, :])
                mv = q1s.tile([128, 2], F32)
                nc.vector.bn_aggr(out=mv[:], in_=stats[:])
                sd = q1s.tile([128, 1], F32, tag="sd")
                nc.scalar.activation(out=sd[:], in_=mv[:, 1:2], func=AF.Sqrt,
                                     bias=eps_sb[:], scale=1.0)
                rstd = q1s.tile([128, 1], F32)
                nc.vector.reciprocal(rstd[:], sd[:])
                h2 = q1.tile([128, C], BF16, tag="h2")
                nc.vector.tensor_scalar(out=h2[:], in0=x2[:],
                                        scalar1=mv[:, 0:1], scalar2=rstd[:],
                                        op0=ALU.subtract, op1=ALU.mult)
                for c in range(8):
                    pt = q1pt.tile([128, 128], BF16)
                    nc.tensor.transpose(pt[:], h2[:, c * 128:(c + 1) * 128],
                                        identb[:])
                    nc.vector.tensor_copy(h2T[:, c, tt * 128:(tt + 1) * 128],
                                          pt[:])

        # ---- P2 + P3 interleaved: c_fc + activation for weight-chunk hc, then
        # immediately the fc2 contribution of those 512 hidden dims to the
        # co=0 output half (4 PSUM banks held); the co=1 half runs as a
        # second fc2-only pass once all of actT exists (weights re-streamed
        # per half, same total bytes).
        with tc.tile_pool(name="q2w", bufs=2) as q2w, \
             tc.tile_pool(name="q3w", bufs=2) as q3w, \
             tc.tile_pool(name="q3o", bufs=3) as q3o, \
             tc.tile_pool(name="q2psum", bufs=3, space="PSUM") as q2p, \
             tc.tile_pool(name="q3psum", bufs=1, space="PSUM") as q3p:
            for co in range(2):
                po_tiles = [q3p.tile([128, 512], F32, tag=f"o{tt}",
                                     name=f"po{tt}{co}")
                            for tt in range(n_ttiles)]
                for hc in range(8):
                    if co == 0:
                        wt = q2w.tile([128, 8, 512], BF16)
                        nc.sync.dma_start(out=wt[:], in_=fcw_d[hc])
                        for ht in range(4):
                            pu = q2p.tile([128, TS], F32)
                            for c in range(8):
                                nc.tensor.matmul(
                                    pu[:], wt[:, c, ht * 128:(ht + 1) * 128],
                                    h2T[:, c, :], start=(c == 0), stop=(c == 7))
                            hi = hc * 4 + ht
                            nc.scalar.activation(
                                out=actT[:, hi, :], in_=pu[:],
                                func=AF.Derivative_Erf,
                                bias=ab_sb[:, hi:hi + 1], scale=s_act)
                    w2 = q3w.tile([128, 4, 512], BF16, tag="w2")
                    nc.sync.dma_start(out=w2[:], in_=fc2w_d[hc, co])
                    for tt in range(n_ttiles):
                        for k4 in range(4):
                            k = hc * 4 + k4
                            nc.tensor.matmul(
                                po_tiles[tt][:],
                                actT[:, k, tt * 128:(tt + 1) * 128],
                                w2[:, k4, :],
                                start=(hc == 0 and k4 == 0),
                                stop=(hc == 7 and k4 == 3),
                                skip_group_check=True)
                for tt in range(n_ttiles):
                    ot = q3o.tile([128, 512], F32)
                    nc.vector.tensor_add(ot[:], po_tiles[tt][:],
                                         x2pb[:, tt, co * 512:(co + 1) * 512])
                    nc.sync.dma_start(
                        out=out_d[tt * 128:(tt + 1) * 128,
                                  co * 512:(co + 1) * 512], in_=ot[:])
    nc.compile()
    return nc


# --------------------------------------------------------------------------
# Host-side orchestration
# --------------------------------------------------------------------------
_PROG_CACHE = {}


def _get_prog(key, builder, *args):
    if key not in _PROG_CACHE:
        _PROG_CACHE[key] = builder(*args)
    return _PROG_CACHE[key]


def _causal_masks4():
    s = np.arange(128)[:, None]
    t = np.arange(512)[None, :]
    ms = [((s + 128 * m) <= t).astype(np.float32) for m in range(4)]
    m4 = np.stack(ms, axis=1)                          # [128, 4, 512]
    return np.ascontiguousarray(np.concatenate([m4, m4], axis=2))  # dup heads


def _perm(w, tiles, width):
    """[tiles*128, width] -> [128, tiles, width] (partition-major for DMA)."""
    return np.ascontiguousarray(w.reshape(tiles, 128, width).transpose(1, 0, 2))


def _bf(a):
    return np.ascontiguousarray(np.asarray(a, dtype=np.float32).astype(BF))


def kernel(x, ln1_w, ln1_b, attn_w, attn_b, proj_w, proj_b,
           ln2_w, ln2_b, fc_w, fc_b, fc2_w, fc2_b,
           mu, sigma, gamma, beta, n_head):
    x = np.asarray(x, dtype=np.float32)
    attn_w = np.asarray(attn_w, dtype=np.float32)
    attn_b = np.asarray(attn_b, dtype=np.float32)
    proj_w = np.asarray(proj_w, dtype=np.float32)
    proj_b = np.asarray(proj_b, dtype=np.float32)
    fc_w = np.asarray(fc_w, dtype=np.float32)
    fc_b = np.asarray(fc_b, dtype=np.float32)
    fc2_w = np.asarray(fc2_w, dtype=np.float32)
    fc2_b = np.asarray(fc2_b, dtype=np.float32)
    ln1_w = np.asarray(ln1_w, dtype=np.float32)
    ln1_b = np.asarray(ln1_b, dtype=np.float32)
    ln2_w = np.asarray(ln2_w, dtype=np.float32)
    ln2_b = np.asarray(ln2_b, dtype=np.float32)
    mu = float(mu)
    sigma = float(sigma)
    gamma = float(gamma)
    beta = float(beta)
    n_head = int(n_head)

    B = x.shape[0]
    assert x.shape == (B, T, C) and B == 2 and n_head == 16

    _install_compile_cache()
    trace = bool(int(os.environ.get("BASS_KERNEL_TRACE", "0")))

    sig = abs(sigma) + 1e-8
    s_act = float(1.0 / (np.sqrt(2.0) * sig))

    # Fold LN affine params into the consuming projection weights (host-side).
    attn_w_eff = attn_w * ln1_w[None, :]
    attn_b_eff = attn_b + attn_w @ ln1_b
    fc_w_eff = fc_w * ln2_w[None, :]
    fc_b_eff = fc_b + fc_w @ ln2_b

    # ---- launch 1 ----
    nc1 = _get_prog(("l1",), build_l1)
    masks = _bf(_causal_masks4())
    vones = _bf(np.ones((128, 1), dtype=np.float32))
    ident = np.eye(128, dtype=np.float32)
    in_maps1 = []
    for c in range(N_CORES):
        b, hg = c // 4, c % 4
        q_rows = attn_w_eff[hg * 256:(hg + 1) * 256] * 0.125
        k_rows = attn_w_eff[C + hg * 256:C + (hg + 1) * 256]
        v_rows = attn_w_eff[2 * C + hg * 256:2 * C + (hg + 1) * 256]
        wqk = np.concatenate([q_rows, k_rows], axis=0)   # [512, 1024]
        bqk = np.concatenate([attn_b_eff[hg * 256:(hg + 1) * 256] * 0.125,
                              attn_b_eff[C + hg * 256:C + (hg + 1) * 256]])
        bv = attn_b_eff[2 * C + hg * 256:2 * C + (hg + 1) * 256]
        m = {
            "xT": _bf(_perm(np.ascontiguousarray(x[b].T), 8, T)
                      .reshape(128, 8, 4, 512).transpose(2, 0, 1, 3)),
            "wqkT": _bf(_perm(np.ascontiguousarray(wqk.T), 8, 512)),
            "wvT": _bf(_perm(np.ascontiguousarray(v_rows.T), 8, 256)),
            "nws": np.ascontiguousarray(-wqk.sum(axis=1).reshape(1, 4, 128)),
            "bqk": np.ascontiguousarray(bqk.reshape(1, 4, 128)),
            "bv": np.ascontiguousarray(bv[None, :]),
            "nwsv": np.ascontiguousarray(-v_rows.sum(axis=1)[None, :]),
            "masks": masks,
            "vones": vones,
            "ident": ident,
        }
        in_maps1.append(m)
    res1 = run_bass_kernel_spmd(nc1, in_maps1, list(range(N_CORES)), trace=trace)
    if res1.exec_time_ns is not None:
        LAST_EXEC_NS["l1"] = res1.exec_time_ns
    # yT [128, 2, 2048]: head h local = 2*cl + (po//64); reorder to [256, 2048]
    ystrips = []
    for c in range(N_CORES):
        yt = np.asarray(res1.results[c]["yT"])
        ystrips.append(np.ascontiguousarray(
            yt.reshape(2, 64, 2, T).transpose(2, 0, 1, 3).reshape(256, T)))

    # ---- launch 2 ----
    nc2 = _get_prog(("l2", s_act), build_l2, s_act)
    fc2w_eff = (gamma * np.sqrt(np.pi) / 2.0 * fc2_w).T      # [4096, 1024]
    fb2_eff = fc2_b + beta * fc2_w.sum(axis=1)
    abias = ((fc_b_eff - mu) * s_act).reshape(32, 128).T     # [128, 32]
    fcwT_p = _perm(np.ascontiguousarray(fc_w_eff.T), 8, HID)      # [128,8,4096]
    fcw_chunks = _bf(
        fcwT_p.reshape(128, 8, 8, 512).transpose(2, 0, 1, 3))     # [8,128,8,512]
    fc2wT_p = _perm(np.ascontiguousarray(fc2w_eff), 32, C)        # [128,32,1024]
    fc2w_chunks = _bf(                                      # [8,2,128,4,512]
        fc2wT_p.reshape(128, 8, 4, 2, 512).transpose(1, 3, 0, 2, 4))
    pwT = _bf(_perm(np.ascontiguousarray(proj_w.T), 8, C))        # [128,8,1024]
    identb = _bf(np.eye(128, dtype=np.float32))
    in_maps2 = []
    for c in range(N_CORES):
        b, slc = c // 4, c % 4
        t0 = slc * TS
        yin = np.concatenate(
            [ystrips[b * 4 + g][:, t0:t0 + TS] for g in range(4)], axis=0)
        m = {
            "yin": np.ascontiguousarray(
                yin.reshape(8, 128, TS).transpose(1, 0, 2)),
            "pwT": pwT,
            "xs": np.ascontiguousarray(x[b, t0:t0 + TS]),
            "pb": proj_b[None, :],
            "fb2": np.ascontiguousarray(fb2_eff[None, :]),
            "abias": np.ascontiguousarray(abias),
            "fcwT": fcw_chunks,
            "fc2wT": fc2w_chunks,
            "identb": identb,
        }
        in_maps2.append(m)
    res2 = run_bass_kernel_spmd(nc2, in_maps2, list(range(N_CORES)), trace=trace)
    if res2.exec_time_ns is not None:
        LAST_EXEC_NS["l2"] = res2.exec_time_ns

    out = np.empty((B, T, C), dtype=np.float32)
    for c in range(N_CORES):
        b, slc = c // 4, c % 4
        out[b, slc * TS:(slc + 1) * TS] = res2.results[c]["out"]
    return out


# revision 28
# speedup vs baseline: 1.4450x; 1.0166x over previous
"""Trainium2 Bass kernel for a dense transformer block (B=2, T=2048, C=1024, nh=16, H=4096).

Two SPMD launches over 8 NeuronCores (no device collectives):

  Launch 1 (head-parallel attention): cores 0-3 <- batch 0, cores 4-7 <- batch 1;
    each core does 4 heads over the full sequence. x arrives pre-transposed
    (feature-major) in bf16. LN1 statistics via ones-matmuls; the -mu*wsum and
    +bias LN terms are folded into the QKV PSUM accumulation as two rank-1
    matmuls, so the epilogue is a single VectorE multiply by the broadcast
    rstd. Q/K/V and attention run in bf16 (fp32 PSUM accumulate). The paired
    heads (even/odd) live in partition halves of the same feature tiles, so
    QK^T runs as 4 concurrent quadrant matmuls (tile_position packing, 2x).
    V carries a ones column so the softmax denominator falls out of the AV
    matmul; the divide happens right after each AV accumulation finishes.
    Output: un-projected attention y (feature-major, bf16) [128, 2, 2048].

  Host: pure re-slicing (no arithmetic on activations).

  Launch 2 (token-parallel): each core takes a 512-token slice: c_proj over
    the concatenated head outputs (contracts over all 1024 y-features, so no
    separate partial reduction is needed) + residual + proj_b -> LN2
    (bn_stats) -> transpose -> c_fc (bf16 weights) -> gaussian activation as a
    single Derivative_Erf pass (d/dx erf = 2/sqrt(pi) e^{-x^2}; sqrt(pi)/2,
    gamma, beta, mu, sigma and biases are folded host-side) -> c_fc2 (bf16)
    -> + residual -> output slice.
"""

import hashlib
import os
import shutil
from contextlib import ExitStack

import numpy as np
import ml_dtypes

import concourse.bass as bass
import concourse.tile as tile
from concourse import bacc, mybir
from concourse.bass_utils import run_bass_kernel_spmd

F32 = mybir.dt.float32
F32R = mybir.dt.float32r
BF16 = mybir.dt.bfloat16
AF = mybir.ActivationFunctionType
ALU = mybir.AluOpType
BF = ml_dtypes.bfloat16

N_CORES = 8
T = 2048          # tokens per batch
C = 1024          # model dim
NH_LOC = 4        # heads per core (launch 1)
HS = 64           # head size
HID = 4096        # mlp hidden
TS = 512          # tokens per core (launch 2)

LAST_EXEC_NS = {}  # launch name -> exec_time_ns (filled when tracing enabled)

_CACHE_DIR = "/tmp/neff_cache"


def _install_compile_cache():
    import concourse.bass2jax as b2j

    if getattr(b2j, "_neff_cache_installed", False):
        return
    real = b2j.compile_bir_kernel

    def cached(bir_json, tmpdir, neff_name="file.neff"):
        os.makedirs(_CACHE_DIR, exist_ok=True)
        h = hashlib.sha256(bir_json).hexdigest()
        cpath = os.path.join(_CACHE_DIR, h + ".neff")
        out = os.path.join(tmpdir, neff_name)
        if os.path.exists(cpath):
            shutil.copyfile(cpath, out)
            return out
        res = real(bir_json, tmpdir, neff_name)
        shutil.copyfile(res, cpath)
        return res

    b2j.compile_bir_kernel = cached
    b2j._neff_cache_installed = True


# --------------------------------------------------------------------------
# Launch 1: LN1 + QKV + causal attention (4 heads), un-projected y out
# --------------------------------------------------------------------------
def build_l1():
    nc = bacc.Bacc("TRN2", target_bir_lowering=False, debug=False,
                   num_devices=N_CORES)
    xT_d = nc.dram_tensor("xT", [4, 128, 8, 512], BF16, kind="ExternalInput")
    wqk_d = nc.dram_tensor("wqkT", [2, 128, 8, 256], BF16, kind="ExternalInput")
    wv_d = nc.dram_tensor("wvT", [128, 8, 256], BF16, kind="ExternalInput")
    nws_d = nc.dram_tensor("nws", [1, 4, 128], F32R, kind="ExternalInput")
    bqk_d = nc.dram_tensor("bqk", [1, 4, 128], F32R, kind="ExternalInput")
    bv_d = nc.dram_tensor("bv", [1, 256], F32, kind="ExternalInput")
    nwsv_d = nc.dram_tensor("nwsv", [1, 256], F32, kind="ExternalInput")
    mask_d = nc.dram_tensor("masks", [128, 4, 1024], BF16, kind="ExternalInput")
    vones_d = nc.dram_tensor("vones", [128, 1], BF16, kind="ExternalInput")
    id_d = nc.dram_tensor("ident", [128, 128], F32, kind="ExternalInput")
    yT_d = nc.dram_tensor("yT", [128, 2, T], BF16, kind="ExternalOutput")

    n_tc = T // 512              # 4 query chunks

    with tile.TileContext(nc) as tc, ExitStack() as ctx:
        consts = ctx.enter_context(tc.tile_pool(name="consts", bufs=1))
        vones_sb = consts.tile([128, 1], BF16)
        nc.sync.dma_start(out=vones_sb[:], in_=vones_d[:])
        eps_sb = consts.tile([128, 1], F32)
        nc.vector.memset(eps_sb[:], 1e-5)
        nws_sb = consts.tile([1, 4, 128], F32R)
        nc.sync.dma_start(out=nws_sb[:], in_=nws_d[:])
        bqk_sb = consts.tile([1, 4, 128], F32R)
        nc.sync.dma_start(out=bqk_sb[:], in_=bqk_d[:])
        ident = consts.tile([128, 128], F32)
        nc.sync.dma_start(out=ident[:], in_=id_d[:])
        bv_row = consts.tile([1, 256], F32)
        nc.sync.dma_start(out=bv_row[:], in_=bv_d[:])
        bv_b = consts.tile([128, 256], F32)
        nc.gpsimd.partition_broadcast(bv_b[:], bv_row[:])
        nwsv_row = consts.tile([1, 256], F32)
        nc.sync.dma_start(out=nwsv_row[:], in_=nwsv_d[:])
        nwsv_b = consts.tile([128, 256], F32)
        nc.gpsimd.partition_broadcast(nwsv_b[:], nwsv_row[:])

        big2 = ctx.enter_context(tc.tile_pool(name="big2", bufs=1))
        qkT = big2.tile([128, 4, T], BF16)     # Q feats (tiles 0,1), K feats (2,3)
        v_sb = big2.tile([128, 16, NH_LOC, 65], BF16)  # col 64 = ones
        yT = big2.tile([128, 2, T], BF16)
        nc.vector.memset(v_sb[:, :, :, 64:65], 1.0)

        p1sl = ctx.enter_context(tc.tile_pool(name="p1sl", bufs=2))
        slab0 = p1sl.tile([128, 8, 512], BF16, tag="slab")
        nc.sync.dma_start(out=slab0[:, 0:4, :], in_=xT_d[0, :, 0:4, :])
        nc.sync.dma_start(out=slab0[:, 4:8, :], in_=xT_d[0, :, 4:8, :])

        wpool = ctx.enter_context(tc.tile_pool(name="wpool", bufs=1))
        wqk_sb = wpool.tile([128, 8, 512], BF16)
        nc.sync.dma_start(out=wqk_sb[:, :, 0:256], in_=wqk_d[0])
        nc.sync.dma_start(out=wqk_sb[:, :, 256:512], in_=wqk_d[1])
        wv_sb = wpool.tile([128, 8, 256], BF16)
        nc.sync.dma_start(out=wv_sb[:], in_=wv_d[:])

        # ---- P1 per 512-token chunk: raw QKV matmuls on un-normalized xT;
        # LN folded in as rank-1 PSUM corrections + one epilogue multiply:
        #   qkv[f,t] = r[t]*(raw[f,t] - mu[t]*wsum[f] + sd[t]*b[f])
        with tc.tile_pool(name="p1", bufs=3) as p1, \
             tc.tile_pool(name="p1sq", bufs=3) as p1sq, \
             tc.tile_pool(name="p1b", bufs=2) as p1b, \
             tc.tile_pool(name="p1c", bufs=2) as p1c, \
             tc.tile_pool(name="p1r", bufs=8) as p1r, \
             tc.tile_pool(name="p1psum", bufs=1, space="PSUM") as p1p, \
             tc.tile_pool(name="ptp", bufs=2, space="PSUM") as ptp, \
             tc.tile_pool(name="p2psum", bufs=2, space="PSUM") as p2p:
            for tch in range(n_tc):
                sl = slice(tch * 512, (tch + 1) * 512)
                if tch == 0:
                    slab = slab0
                else:
                    slab = p1sl.tile([128, 8, 512], BF16, tag="slab")
                    nc.sync.dma_start(out=slab[:, 0:4, :],
                                      in_=xT_d[tch, :, 0:4, :])
                    nc.sync.dma_start(out=slab[:, 4:8, :],
                                      in_=xT_d[tch, :, 4:8, :])
                ps_sum = p1p.tile([1, 512], F32, tag="s")
                ps_sq = p1p.tile([1, 512], F32, tag="q")
                for c in range(8):
                    sq = p1sq.tile([128, 512], BF16, tag="sq")
                    nc.scalar.activation(out=sq[:], in_=slab[:, c, :],
                                         func=AF.Square)
                    nc.tensor.matmul(ps_sum[:], vones_sb[:], slab[:, c, :],
                                     start=(c == 0), stop=(c == 7),
                                     skip_group_check=True)
                    nc.tensor.matmul(ps_sq[:], vones_sb[:], sq[:],
                                     start=(c == 0), stop=(c == 7),
                                     skip_group_check=True)
                # row math: mu, sd, rstd (rows live on partition 0).
                # mrow/sdr are F32R because they feed matmuls as rhs.
                srow = p1r.tile([1, 512], F32, tag="row")
                nc.vector.tensor_copy(srow[:], ps_sum[:])
                qrow = p1r.tile([1, 512], F32, tag="row")
                nc.vector.tensor_copy(qrow[:], ps_sq[:])
                mrow = p1r.tile([1, 512], F32R, tag="rowr")
                nc.vector.tensor_scalar(out=mrow[:], in0=srow[:], scalar1=1.0 / C,
                                        scalar2=None, op0=ALU.mult)
                msq = p1r.tile([1, 512], F32, tag="row")
                nc.vector.tensor_mul(msq[:], mrow[:], mrow[:])
                vrow = p1r.tile([1, 512], F32, tag="row")
                nc.vector.scalar_tensor_tensor(out=vrow[:], in0=qrow[:],
                                               scalar=1.0 / C, in1=msq[:],
                                               op0=ALU.mult, op1=ALU.subtract)
                sdr = p1r.tile([1, 512], F32R, tag="rowr")
                nc.scalar.activation(out=sdr[:], in_=vrow[:], func=AF.Sqrt,
                                     bias=eps_sb[0:1], scale=1.0)
                rrow = p1r.tile([1, 512], F32, tag="row")
                nc.vector.reciprocal_approx_fast(rrow[:],
                                                 sdr[:].bitcast(F32))
                rmurow = p1r.tile([1, 512], F32, tag="row")
                nc.vector.tensor_mul(rmurow[:], rrow[:], mrow[:])
                rb = p1b.tile([128, 512], F32, tag="rb")
                nc.gpsimd.partition_broadcast(rb[:], rrow[:])
                # token-major per-partition columns of r and r*mu (for V)
                rcol = p1c.tile([128, 4], F32, tag="rcol")
                rmucol = p1c.tile([128, 4], F32, tag="rmucol")
                for j in range(4):
                    tp1 = ptp.tile([128, 1], F32, tag="tp")
                    nc.tensor.transpose(tp1[:], rrow[0:1, j * 128:(j + 1) * 128],
                                        ident[0:1, 0:1])
                    nc.vector.tensor_copy(rcol[:, j:j + 1], tp1[:])
                    tp2 = ptp.tile([128, 1], F32, tag="tp")
                    nc.tensor.transpose(tp2[:], rmurow[0:1, j * 128:(j + 1) * 128],
                                        ident[0:1, 0:1])
                    nc.vector.tensor_copy(rmucol[:, j:j + 1], tp2[:])
                # raw Q/K projections + rank-1 LN corrections + epilogue
                for f in range(4):
                    ps = p2p.tile([128, 512], F32, tag="qk")
                    for c in range(8):
                        nc.tensor.matmul(
                            ps[:], wqk_sb[:, c, f * 128:(f + 1) * 128],
                            slab[:, c, :], start=(c == 0), stop=False,
                            skip_group_check=True)
                    nc.tensor.matmul(ps[:], nws_sb[:, f, :], mrow[:],
                                     start=False, stop=False,
                                     skip_group_check=True)
                    nc.tensor.matmul(ps[:], bqk_sb[:, f, :], sdr[:],
                                     start=False, stop=True,
                                     skip_group_check=True)
                    nc.vector.tensor_mul(qkT[:, f, sl], ps[:], rb[:])
                # raw V + LN epilogue (token-major)
                for tt4 in range(4):
                    tt = tch * 4 + tt4
                    ps = p2p.tile([128, 256], F32, tag="v")
                    for c in range(8):
                        nc.tensor.matmul(
                            ps[:], slab[:, c, tt4 * 128:(tt4 + 1) * 128],
                            wv_sb[:, c, :], start=(c == 0), stop=(c == 7))
                    corr = p1.tile([128, 256], F32, tag="corr")
                    nc.vector.scalar_tensor_tensor(
                        out=corr[:], in0=nwsv_b[:],
                        scalar=rmucol[:, tt4:tt4 + 1], in1=bv_b[:],
                        op0=ALU.mult, op1=ALU.add)
                    nc.vector.scalar_tensor_tensor(
                        out=v_sb[:, tt, :, 0:64],
                        in0=ps[:].rearrange("p (h d) -> p h d", h=NH_LOC),
                        scalar=rcol[:, tt4:tt4 + 1],
                        in1=corr[:].rearrange("p (h d) -> p h d", h=NH_LOC),
                        op0=ALU.mult, op1=ALU.add)

        # ---- P3: attention; paired heads (even/odd partition halves) share
        # quadrant-packed QK matmuls and one wide exp per (s, tcx).
        with tc.tile_pool(name="p3consts", bufs=1) as p3c, \
             tc.tile_pool(name="p3a", bufs=4) as p3a, \
             tc.tile_pool(name="p3y", bufs=4) as p3y, \
             tc.tile_pool(name="p3d", bufs=4) as p3d, \
             tc.tile_pool(name="p3sc", bufs=2, space="PSUM") as p3sc, \
             tc.tile_pool(name="p3py", bufs=1, space="PSUM") as p3py:
            mask_sb = p3c.tile([128, 4, 1024], BF16)
            nc.sync.dma_start(out=mask_sb[:], in_=mask_d[:])
            for g in range(2):
                hA, hB = 2 * g, 2 * g + 1
                qf = g
                kf = 2 + g
                for pair in ((0, 1), (2, 3)):
                    smax = 4 * pair[1] + 4
                    pys = {}
                    for tcx in pair:
                        for h, po in ((hA, 0), (hB, 64)):
                            pys[(h, tcx)] = p3py.tile(
                                [65, 512], F32, tag=f"py{h % 2}{tcx % 2}",
                                name=f"py{h}_{tcx}")

                    def emit_qk(s):
                        tiles = []
                        for tcx in pair:
                            if s > 4 * tcx + 3:
                                continue
                            qsl = slice(tcx * 512, (tcx + 1) * 512)
                            sc = p3sc.tile([128, 1024], F32, tag="sc",
                                           name=f"sc{g}_{s}_{tcx}")
                            k0 = s * 128
                            # issue order pairs disjoint quadrants so the
                            # sub-arrays overlap: (A-lo | B-hi), (A-hi | B-lo)
                            for hj, kj in ((0, 0), (1, 1), (0, 1), (1, 0)):
                                po = hj * 64
                                nc.tensor.matmul(
                                    sc[kj * 64:(kj + 1) * 64,
                                       hj * 512:(hj + 1) * 512],
                                    qkT[po:po + 64, kf,
                                        k0 + kj * 64:k0 + (kj + 1) * 64],
                                    qkT[po:po + 64, qf, qsl],
                                    start=True, stop=True,
                                    skip_group_check=True,
                                    tile_position=(po, kj * 64))
                            at = p3a.tile([128, 1024], BF16, tag="at",
                                          name=f"at{g}_{s}_{tcx}")
                            nc.scalar.activation(out=at[:], in_=sc[:],
                                                 func=AF.Exp)
                            if tcx == s // 4:
                                atm = p3a.tile([128, 1024], BF16, tag="atm",
                                               name=f"atm{g}_{s}_{tcx}")
                                nc.vector.tensor_mul(atm[:], at[:],
                                                     mask_sb[:, s % 4, :])
                                at = atm
                            tiles.append((tcx, at))
                        return tiles

                    cur = emit_qk(0)
                    for s in range(smax):
                        nxt = emit_qk(s + 1) if s + 1 < smax else []
                        for tcx, at in cur:
                            qsl = slice(tcx * 512, (tcx + 1) * 512)
                            for h, po in ((hA, 0), (hB, 64)):
                                py = pys[(h, tcx)]
                                nc.tensor.matmul(
                                    py[:], v_sb[:, s, h, :],
                                    at[:, (po // 64) * 512:(po // 64) * 512 + 512],
                                    start=(s == 0), stop=(s == 4 * tcx + 3),
                                    skip_group_check=True)
                                if s == 4 * tcx + 3:
                                    dcp = p3y.tile([65, 512], F32, tag="dcp",
                                                   name=f"dcp{h}_{tcx}")
                                    nc.vector.tensor_copy(dcp[64:65, :],
                                                          py[64:65, :])
                                    dn = p3d.tile([1, 512], F32, tag="dn",
                                                  name=f"dn{h}_{tcx}")
                                    nc.sync.dma_start(out=dn[:],
                                                      in_=dcp[64:65, :])
                                    dnr = p3d.tile([1, 512], F32, tag="dnr",
                                                   name=f"dnr{h}_{tcx}")
                                    nc.vector.reciprocal_approx_fast(
                                        dnr[:], dn[:])
                                    db = p3d.tile([64, 512], F32, tag="db",
                                                  name=f"db{h}_{tcx}")
                                    nc.gpsimd.partition_broadcast(
                                        db[:], dnr[:])
                                    nc.vector.tensor_mul(
                                        yT[po:po + 64, g, qsl],
                                        py[0:64, :], db[:])
                        cur = nxt
                    # stream finished token range of this head-pair out
                    psl = slice(pair[0] * 512, (pair[1] + 1) * 512)
                    nc.sync.dma_start(out=yT_d[:, g, psl], in_=yT[:, g, psl])
    nc.compile()
    return nc


# --------------------------------------------------------------------------
# Launch 2: c_proj + residual + LN2 + MLP + residual
# --------------------------------------------------------------------------
def build_l2(s_act: float):
    nc = bacc.Bacc("TRN2", target_bir_lowering=False, debug=False,
                   num_devices=N_CORES)
    yin_d = nc.dram_tensor("yin", [128, 8, 512], BF16, kind="ExternalInput")
    pw_d = nc.dram_tensor("pwT", [128, 8, 1024], BF16, kind="ExternalInput")
    xs_d = nc.dram_tensor("xs", [TS, C], F32, kind="ExternalInput")
    pb_d = nc.dram_tensor("pb", [1, C], F32, kind="ExternalInput")
    fb2_d = nc.dram_tensor("fb2", [1, C], F32, kind="ExternalInput")
    ab_d = nc.dram_tensor("abias", [128, 32], F32, kind="ExternalInput")
    fcw_d = nc.dram_tensor("fcwT", [8, 128, 8, 512], BF16, kind="ExternalInput")
    fc2w_d = nc.dram_tensor("fc2wT", [8, 2, 128, 4, 512], BF16,
                            kind="ExternalInput")
    idb_d = nc.dram_tensor("identb", [128, 128], BF16, kind="ExternalInput")
    out_d = nc.dram_tensor("out", [TS, C], F32, kind="ExternalOutput")

    n_ttiles = TS // 128    # 4

    with tile.TileContext(nc) as tc, ExitStack() as ctx:
        consts = ctx.enter_context(tc.tile_pool(name="consts", bufs=1))
        identb = consts.tile([128, 128], BF16)
        nc.sync.dma_start(out=identb[:], in_=idb_d[:])
        eps_sb = consts.tile([128, 1], F32)
        nc.vector.memset(eps_sb[:], 1e-5)
        pb_row = consts.tile([1, C], F32)
        nc.sync.dma_start(out=pb_row[:], in_=pb_d[:])
        pb_b = consts.tile([128, C], F32)
        nc.gpsimd.partition_broadcast(pb_b[:], pb_row[:])
        fb2_row = consts.tile([1, C], F32)
        nc.sync.dma_start(out=fb2_row[:], in_=fb2_d[:])
        fb2_b = consts.tile([128, C], F32)
        nc.gpsimd.partition_broadcast(fb2_b[:], fb2_row[:])
        ab_sb = consts.tile([128, 32], F32)
        nc.sync.dma_start(out=ab_sb[:], in_=ab_d[:])

        big = ctx.enter_context(tc.tile_pool(name="big", bufs=1))
        h2T = big.tile([128, 8, TS], BF16)         # 8KB/p
        x2pb = big.tile([128, n_ttiles, C], F32)   # x2 + fc2 bias, 16KB/p
        actT = big.tile([128, 32, TS], BF16)       # 32KB/p
        yin_sb = big.tile([128, 8, 512], BF16)
        nc.sync.dma_start(out=yin_sb[:, 0:4, :], in_=yin_d[:, 0:4, :])
        nc.sync.dma_start(out=yin_sb[:, 4:8, :], in_=yin_d[:, 4:8, :])
        pw_sb = big.tile([128, 8, 1024], BF16)
        nc.sync.dma_start(out=pw_sb[:, 0:4, :], in_=pw_d[:, 0:4, :])
        nc.sync.dma_start(out=pw_sb[:, 4:8, :], in_=pw_d[:, 4:8, :])

        # ---- P1: c_proj + residual + proj bias, LN2, transpose ----
        with tc.tile_pool(name="q1", bufs=3) as q1, \
             tc.tile_pool(name="q1s", bufs=4) as q1s, \
             tc.tile_pool(name="q1psum", bufs=2, space="PSUM") as q1p, \
             tc.tile_pool(name="q1pt", bufs=2, space="PSUM") as q1pt:
            for tt in range(n_ttiles):
                x2 = q1.tile([128, C], F32, tag="x2")
                px2 = [q1p.tile([128, 512], F32, tag=f"px{ch}", name=f"px{tt}_{ch}")
                       for ch in range(2)]
                for k in range(8):
                    for ch in range(2):
                        nc.tensor.matmul(
                            px2[ch][:], yin_sb[:, k, tt * 128:(tt + 1) * 128],
                            pw_sb[:, k, ch * 512:(ch + 1) * 512],
                            start=(k == 0), stop=(k == 7),
                            skip_group_check=True)
                xst = q1.tile([128, C], F32, tag="xs")
                nc.sync.dma_start(out=xst[:], in_=xs_d[tt * 128:(tt + 1) * 128, :])
                xpb = q1.tile([128, C], F32, tag="xpb")
                nc.gpsimd.tensor_add(xpb[:], xst[:], pb_b[:])
                for ch in range(2):
                    csl = slice(ch * 512, (ch + 1) * 512)
                    nc.vector.tensor_add(x2[:, csl], px2[ch][:], xpb[:, csl])
                nc.vector.tensor_add(x2pb[:, tt, :], x2[:], fb2_b[:])
                stats = q1s.tile([128, 2, 6], F32)
                x2g = x2[:].rearrange("p (g d) -> p g d", g=2)
                nc.vector.bn_stats(out=stats[:, 0, :], in_=x2g[:, 0, :])
                nc.vector.bn_stats(out=stats[:, 1, :], in_=x2g[:, 1, :])
                mv = q1s.tile([128, 2], F32)
                nc.vector.bn_aggr(out=mv[:], in_=stats[:])
                sd = q1s.tile([128, 1], F32, tag="sd")
                nc.scalar.activation(out=sd[:], in_=mv[:, 1:2], func=AF.Sqrt,
                                     bias=eps_sb[:], scale=1.0)
                rstd = q1s.tile([128, 1], F32)
                nc.vector.reciprocal(rstd[:], sd[:])
                h2 = q1.tile([128, C], BF16, tag="h2")
                nc.vector.tensor_scalar(out=h2[:], in0=x2[:],
                                        scalar1=mv[:, 0:1], scalar2=rstd[:],
                                        op0=ALU.subtract, op1=ALU.mult)
                for c in range(8):
                    pt = q1pt.tile([128, 128], BF16)
                    nc.tensor.transpose(pt[:], h2[:, c * 128:(c + 1) * 128],
                                        identb[:])
                    nc.vector.tensor_copy(h2T[:, c, tt * 128:(tt + 1) * 128],
                                          pt[:])

        # ---- P2 + P3 interleaved: c_fc + activation for weight-chunk hc, then
        # immediately the fc2 contribution of those 512 hidden dims to the
        # co=0 output half (4 PSUM banks held); the co=1 half runs as a
        # second fc2-only pass once all of actT exists (weights re-streamed
        # per half, same total bytes).
        with tc.tile_pool(name="q2w", bufs=2) as q2w, \
             tc.tile_pool(name="q3w", bufs=2) as q3w, \
             tc.tile_pool(name="q3o", bufs=3) as q3o, \
             tc.tile_pool(name="q2psum", bufs=3, space="PSUM") as q2p, \
             tc.tile_pool(name="q3psum", bufs=1, space="PSUM") as q3p:
            for co in range(2):
                po_tiles = [q3p.tile([128, 512], F32, tag=f"o{tt}",
                                     name=f"po{tt}{co}")
                            for tt in range(n_ttiles)]
                for hc in range(8):
                    if co == 0:
                        wt = q2w.tile([128, 8, 512], BF16)
                        nc.sync.dma_start(out=wt[:], in_=fcw_d[hc])
                        for ht in range(4):
                            pu = q2p.tile([128, TS], F32)
                            for c in range(8):
                                nc.tensor.matmul(
                                    pu[:], wt[:, c, ht * 128:(ht + 1) * 128],
                                    h2T[:, c, :], start=(c == 0), stop=(c == 7))
                            hi = hc * 4 + ht
                            nc.scalar.activation(
                                out=actT[:, hi, :], in_=pu[:],
                                func=AF.Derivative_Erf,
                                bias=ab_sb[:, hi:hi + 1], scale=s_act)
                    w2 = q3w.tile([128, 4, 512], BF16, tag="w2")
                    nc.sync.dma_start(out=w2[:], in_=fc2w_d[hc, co])
                    for tt in range(n_ttiles):
                        for k4 in range(4):
                            k = hc * 4 + k4
                            nc.tensor.matmul(
                                po_tiles[tt][:],
                                actT[:, k, tt * 128:(tt + 1) * 128],
                                w2[:, k4, :],
                                start=(hc == 0 and k4 == 0),
                                stop=(hc == 7 and k4 == 3),
                                skip_group_check=True)
                for tt in range(n_ttiles):
                    ot = q3o.tile([128, 512], F32)
                    nc.vector.tensor_add(ot[:], po_tiles[tt][:],
                                         x2pb[:, tt, co * 512:(co + 1) * 512])
                    nc.sync.dma_start(
                        out=out_d[tt * 128:(tt + 1) * 128,
                                  co * 512:(co + 1) * 512], in_=ot[:])
    nc.compile()
    return nc


# --------------------------------------------------------------------------
# Host-side orchestration
# --------------------------------------------------------------------------
_PROG_CACHE = {}


def _get_prog(key, builder, *args):
    if key not in _PROG_CACHE:
        _PROG_CACHE[key] = builder(*args)
    return _PROG_CACHE[key]


def _causal_masks4():
    s = np.arange(128)[:, None]
    t = np.arange(512)[None, :]
    ms = [((s + 128 * m) <= t).astype(np.float32) for m in range(4)]
    m4 = np.stack(ms, axis=1)                          # [128, 4, 512]
    return np.ascontiguousarray(np.concatenate([m4, m4], axis=2))  # dup heads


def _perm(w, tiles, width):
    """[tiles*128, width] -> [128, tiles, width] (partition-major for DMA)."""
    return np.ascontiguousarray(w.reshape(tiles, 128, width).transpose(1, 0, 2))


def _bf(a):
    return np.ascontiguousarray(np.asarray(a, dtype=np.float32).astype(BF))


def kernel(x, ln1_w, ln1_b, attn_w, attn_b, proj_w, proj_b,
           ln2_w, ln2_b, fc_w, fc_b, fc2_w, fc2_b,
           mu, sigma, gamma, beta, n_head):
    x = np.asarray(x, dtype=np.float32)
    attn_w = np.asarray(attn_w, dtype=np.float32)
    attn_b = np.asarray(attn_b, dtype=np.float32)
    proj_w = np.asarray(proj_w, dtype=np.float32)
    proj_b = np.asarray(proj_b, dtype=np.float32)
    fc_w = np.asarray(fc_w, dtype=np.float32)
    fc_b = np.asarray(fc_b, dtype=np.float32)
    fc2_w = np.asarray(fc2_w, dtype=np.float32)
    fc2_b = np.asarray(fc2_b, dtype=np.float32)
    ln1_w = np.asarray(ln1_w, dtype=np.float32)
    ln1_b = np.asarray(ln1_b, dtype=np.float32)
    ln2_w = np.asarray(ln2_w, dtype=np.float32)
    ln2_b = np.asarray(ln2_b, dtype=np.float32)
    mu = float(mu)
    sigma = float(sigma)
    gamma = float(gamma)
    beta = float(beta)
    n_head = int(n_head)

    B = x.shape[0]
    assert x.shape == (B, T, C) and B == 2 and n_head == 16

    _install_compile_cache()
    trace = bool(int(os.environ.get("BASS_KERNEL_TRACE", "0")))

    sig = abs(sigma) + 1e-8
    s_act = float(1.0 / (np.sqrt(2.0) * sig))

    # Fold LN affine params into the consuming projection weights (host-side).
    attn_w_eff = attn_w * ln1_w[None, :]
    attn_b_eff = attn_b + attn_w @ ln1_b
    fc_w_eff = fc_w * ln2_w[None, :]
    fc_b_eff = fc_b + fc_w @ ln2_b

    # ---- launch 1 ----
    nc1 = _get_prog(("l1",), build_l1)
    masks = _bf(_causal_masks4())
    vones = _bf(np.ones((128, 1), dtype=np.float32))
    ident = np.eye(128, dtype=np.float32)
    in_maps1 = []
    for c in range(N_CORES):
        b, hg = c // 4, c % 4
        q_rows = attn_w_eff[hg * 256:(hg + 1) * 256] * 0.125
        k_rows = attn_w_eff[C + hg * 256:C + (hg + 1) * 256]
        v_rows = attn_w_eff[2 * C + hg * 256:2 * C + (hg + 1) * 256]
        wqk = np.concatenate([q_rows, k_rows], axis=0)   # [512, 1024]
        bqk = np.concatenate([attn_b_eff[hg * 256:(hg + 1) * 256] * 0.125,
                              attn_b_eff[C + hg * 256:C + (hg + 1) * 256]])
        bv = attn_b_eff[2 * C + hg * 256:2 * C + (hg + 1) * 256]
        wqkT_p = _perm(np.ascontiguousarray(wqk.T), 8, 512)
        m = {
            "xT": _bf(_perm(np.ascontiguousarray(x[b].T), 8, T)
                      .reshape(128, 8, 4, 512).transpose(2, 0, 1, 3)),
            "wqkT": _bf(np.stack([wqkT_p[:, :, 0:256], wqkT_p[:, :, 256:512]])),
            "wvT": _bf(_perm(np.ascontiguousarray(v_rows.T), 8, 256)),
            "nws": np.ascontiguousarray(-wqk.sum(axis=1).reshape(1, 4, 128)),
            "bqk": np.ascontiguousarray(bqk.reshape(1, 4, 128)),
            "bv": np.ascontiguousarray(bv[None, :]),
            "nwsv": np.ascontiguousarray(-v_rows.sum(axis=1)[None, :]),
            "masks": masks,
            "vones": vones,
            "ident": ident,
        }
        in_maps1.append(m)
    res1 = run_bass_kernel_spmd(nc1, in_maps1, list(range(N_CORES)), trace=trace)
    if res1.exec_time_ns is not None:
        LAST_EXEC_NS["l1"] = res1.exec_time_ns
    # yT [128, 2, 2048]: head h local = 2*cl + (po//64); reorder to [256, 2048]
    ystrips = []
    for c in range(N_CORES):
        yt = np.asarray(res1.results[c]["yT"])
        ystrips.append(np.ascontiguousarray(
            yt.reshape(2, 64, 2, T).transpose(2, 0, 1, 3).reshape(256, T)))

    # ---- launch 2 ----
    nc2 = _get_prog(("l2", s_act), build_l2, s_act)
    fc2w_eff = (gamma * np.sqrt(np.pi) / 2.0 * fc2_w).T      # [4096, 1024]
    fb2_eff = fc2_b + beta * fc2_w.sum(axis=1)
    abias = ((fc_b_eff - mu) * s_act).reshape(32, 128).T     # [128, 32]
    fcwT_p = _perm(np.ascontiguousarray(fc_w_eff.T), 8, HID)      # [128,8,4096]
    fcw_chunks = _bf(
        fcwT_p.reshape(128, 8, 8, 512).transpose(2, 0, 1, 3))     # [8,128,8,512]
    fc2wT_p = _perm(np.ascontiguousarray(fc2w_eff), 32, C)        # [128,32,1024]
    fc2w_chunks = _bf(                                      # [8,2,128,4,512]
        fc2wT_p.reshape(128, 8, 4, 2, 512).transpose(1, 3, 0, 2, 4))
    pwT = _bf(_perm(np.ascontiguousarray(proj_w.T), 8, C))        # [128,8,1024]
    identb = _bf(np.eye(128, dtype=np.float32))
    in_maps2 = []
    for c in range(N_CORES):
        b, slc = c // 4, c % 4
        t0 = slc * TS
        yin = np.concatenate(
            [ystrips[b * 4 + g][:, t0:t0 + TS] for g in range(4)], axis=0)
        m = {
            "yin": np.ascontiguousarray(
                yin.reshape(8, 128, TS).transpose(1, 0, 2)),
            "pwT": pwT,
            "xs": np.ascontiguousarray(x[b, t0:t0 + TS]),
            "pb": proj_b[None, :],
            "fb2": np.ascontiguousarray(fb2_eff[None, :]),
            "abias": np.ascontiguousarray(abias),
            "fcwT": fcw_chunks,
            "fc2wT": fc2w_chunks,
            "identb": identb,
        }
        in_maps2.append(m)
    res2 = run_bass_kernel_spmd(nc2, in_maps2, list(range(N_CORES)), trace=trace)
    if res2.exec_time_ns is not None:
        LAST_EXEC_NS["l2"] = res2.exec_time_ns

    out = np.empty((B, T, C), dtype=np.float32)
    for c in range(N_CORES):
        b, slc = c // 4, c % 4
        out[b, slc * TS:(slc + 1) * TS] = res2.results[c]["out"]
    return out
